# revision 1
# baseline (speedup 1.0000x reference)
"""BertSelfAttention kernel for Trainium2 (Bass/Tile), 8-core SPMD.

Full inputs in, full output out. Sharding: core c handles batch b = c//2 and
head-group hg = c%2 (8 of the 16 heads). Each core computes its projections
q/k/v for its 512 output features and full attention for its 8 heads; the
host assembles out[b, :, hg*512:(hg+1)*512] from each core. No collectives.

Problem shapes (hardcoded): B=4, S=2048, H=1024, nh=16, hd=64.
"""

import numpy as np

B, S, H = 4, 2048, 1024
NH, HD = 16, 64
HPC = 8          # heads per core
OC = HPC * HD    # output features per core (512)
NT = S // 128    # n tiles (16)
MC = 512         # m chunk (q positions per attention unit)
NMC = S // MC    # 4
KC = H // 128    # contraction chunks for projections (8)

_CACHE = {}


def _build(has_bv: bool, reps: int = 1, paired: bool = False):
    from contextlib import ExitStack

    import concourse.bass as bass
    from concourse import bacc
    import concourse.tile as tile
    from concourse import mybir
    from concourse.masks import make_identity

    f32 = mybir.dt.float32
    f16 = mybir.dt.float16

    nc = bacc.Bacc(trn_type="TRN2")

    xT = nc.dram_tensor("xt", [H, S], f16, kind="ExternalInput")
    wqT = nc.dram_tensor("wqt", [H, OC], f16, kind="ExternalInput")
    wkT = nc.dram_tensor("wkt", [H, OC], f16, kind="ExternalInput")
    wvT = nc.dram_tensor("wvt", [H, OC], f16, kind="ExternalInput")
    bqT = nc.dram_tensor("bqt", [128, OC // 128], f32, kind="ExternalInput")
    bkT = nc.dram_tensor("bkt", [128, OC // 128], f32, kind="ExternalInput")
    maskT = nc.dram_tensor("maskt", [128, NT], f32, kind="ExternalInput")
    if has_bv:
        bv = nc.dram_tensor("bv", [1, OC], f16, kind="ExternalInput")
    out = nc.dram_tensor("out", [S, OC], f32, kind="ExternalOutput")

    xT_r = xT[:].rearrange("(c p) s -> p c s", p=128)      # [128, KC, S]
    wqT_r = wqT[:].rearrange("(c p) o -> p c o", p=128)    # [128, KC, OC]
    wkT_r = wkT[:].rearrange("(c p) o -> p c o", p=128)
    wvT_r = wvT[:].rearrange("(c p) o -> p c o", p=128)

    with tile.TileContext(nc) as tc, ExitStack() as ctx:
        consts = ctx.enter_context(tc.tile_pool(name="consts", bufs=1))
        ident = consts.tile([128, 128] if paired else [65, 65], f32)
        make_identity(nc, ident)
        if paired:
            identh = consts.tile([128, 128], f16)
            make_identity(nc, identh)
        if paired:
            onesk_sb = consts.tile([128, 1], f16)
            nc.vector.memset(onesk_sb, 1.0)
        mask_sb = consts.tile([128, NT], f32)
        nc.sync.dma_start(out=mask_sb, in_=maskT[:])
        eshift_sb = consts.tile([128, 1], f32)
        nc.vector.memset(eshift_sb, -12.0)
        bq_sb = consts.tile([128, OC // 128], f32)
        nc.sync.dma_start(out=bq_sb, in_=bqT[:])
        bk_sb = consts.tile([128, OC // 128], f32)
        nc.sync.dma_start(out=bk_sb, in_=bkT[:])
        if has_bv:
            bv_sb = consts.tile([1, OC], f16)
            nc.sync.dma_start(out=bv_sb, in_=bv[:])
            ones_sb = consts.tile([1, 128], f16)
            nc.vector.memset(ones_sb, 1.0)

        for rep in range(reps):
            rep_stack = ctx if reps == 1 else ExitStack()
            # Persistent activation tensors
            qkv = ctx.enter_context(tc.tile_pool(name="qkv", bufs=1)) \
                if reps == 1 else rep_stack.enter_context(
                    tc.tile_pool(name="qkv", bufs=1))
            qT_sb = qkv.tile([128, OC // 128, S], f16)   # [128, 4, 2048] o-major
            kT_sb = qkv.tile([128, OC // 128, S], f16)
            v_sb = qkv.tile([128, NT, HPC, 65], f16)     # v + wmask col per head
            # wmask = exp(attention_mask) columns serve as the softmax
            # denominator accumulators; exp(s+mask) = exp(s)*wmask folds the
            # additive mask into the v rows and these columns.
            for gt in range(NT):
                nc.vector.tensor_copy(
                    out=v_sb[:, gt, :, 64:65],
                    in_=mask_sb[:, gt:gt + 1].to_broadcast([128, HPC, 1]))

            with tc.tile_pool(name="xw", bufs=1) as xwpool, \
                 tc.tile_pool(name="exp", bufs=2) as epool, \
                 tc.tile_pool(name="csb", bufs=2) as cpool, \
                 tc.tile_pool(name="osb", bufs=2) as opool, \
                 tc.tile_pool(name="pps", bufs=1, space="PSUM") as ppsum, \
                 tc.tile_pool(name="sps", bufs=2, space="PSUM") as spsum, \
                 tc.tile_pool(name="cps", bufs=2, space="PSUM") as cpsum, \
                 tc.tile_pool(name="tps", bufs=1, space="PSUM") as tpsum:
                wk_sb = xwpool.tile([128, KC, OC], f16)
                nc.sync.dma_start(out=wk_sb, in_=wkT_r)
                xs = []
                for s in range(NMC):
                    t = xwpool.tile([128, KC, MC], f16, name=f"xs{s}")
                    nc.sync.dma_start(out=t,
                                      in_=xT_r[:, :, s * MC:(s + 1) * MC])
                    xs.append(t)
                wq_sb = xwpool.tile([128, KC, OC], f16)
                nc.sync.dma_start(out=wq_sb, in_=wqT_r)
                wv_sb = xwpool.tile([128, KC, OC], f16)
                nc.sync.dma_start(out=wv_sb, in_=wvT_r)

                def kproj(j):
                    for s in range(NMC):
                        ss = slice(s * MC, (s + 1) * MC)
                        psk = ppsum.tile([128, MC], f32, tag="pp", name="psk")
                        for i in range(KC):
                            nc.tensor.matmul(
                                psk, wk_sb[:, i, j * 128:(j + 1) * 128],
                                xs[s][:, i, :], start=(i == 0), stop=(i == KC - 1))
                        nc.vector.tensor_scalar_add(
                            kT_sb[:, j, ss], psk, bk_sb[:, j:j + 1])

                def qproj(j, m):
                    ms = slice(m * MC, (m + 1) * MC)
                    psq = ppsum.tile([128, MC], f32, tag="pp", name="psq")
                    for i in range(KC):
                        nc.tensor.matmul(
                            psq, wq_sb[:, i, j * 128:(j + 1) * 128],
                            xs[m][:, i, :], start=(i == 0), stop=(i == KC - 1))
                    nc.vector.tensor_scalar_add(
                        qT_sb[:, j, ms], psq, bq_sb[:, j:j + 1])

                def vproj():
                    for gt in range(NT):
                        psv = ppsum.tile([128, OC], f32, tag="pp", name="psv")
                        for i in range(KC):
                            nc.tensor.matmul(
                                psv, xs[gt // 4][:, i, (gt % 4) * 128:(gt % 4 + 1) * 128],
                                wv_sb[:, i, :], start=(i == 0),
                                stop=(i == KC - 1 and not has_bv))
                        if has_bv:
                            nc.tensor.matmul(psv, ones_sb, bv_sb,
                                             start=False, stop=True)
                        nc.vector.tensor_scalar_mul(
                            v_sb[:, gt, :, 0:64],
                            psv.rearrange("p (h d) -> p h d", h=HPC),
                            mask_sb[:, gt:gt + 1])

                def scores(j, m):
                    """Scores + exp for head pair j, m-chunk m. Returns exp tiles."""
                    ms = slice(m * MC, (m + 1) * MC)
                    et = [epool.tile([128, NT, MC], f16, tag=f"exp{hh}",
                                     name=f"exp{hh}")
                          for hh in range(2)]
                    for tp in range(NT // 2):    # pairs of n tiles share a psum
                        for hh in range(2):
                            ps = spsum.tile([128, 2, MC], f32, tag="sc", name="ps")
                            for u in range(2):
                                t = 2 * tp + u
                                nc.tensor.matmul(
                                    ps[:, u, :],
                                    kT_sb[hh * 64:(hh + 1) * 64, j,
                                          t * 128:(t + 1) * 128],
                                    qT_sb[hh * 64:(hh + 1) * 64, j, ms],
                                    start=True, stop=True,
                                    tile_position=(hh * 64, 0))
                            # constant shift cancels in softmax normalization;
                            # guards fp16 overflow of exp for scores up to ~23
                            if paired:
                                nc.scalar.activation(
                                    out=et[hh][:, 2 * tp:2 * tp + 2, :],
                                    in_=ps,
                                    func=mybir.ActivationFunctionType.Exp)
                            else:
                                nc.scalar.activation(
                                    out=et[hh][:, 2 * tp:2 * tp + 2, :],
                                    in_=ps,
                                    func=mybir.ActivationFunctionType.Exp,
                                    bias=eshift_sb[:, 0:1])
                    return et

                def ctxpart_paired(j, m, et):
                    gA, gB = 2 * j, 2 * j + 1
                    pc = cpsum.tile([128, MC], f32, tag="ctx")
                    for t in range(NT):
                        nc.tensor.matmul(
                            pc[0:64, :], v_sb[:, t, gA, 0:64], et[0][:, t, :],
                            start=(t == 0), stop=(t == NT - 1),
                            tile_position=(0, 0), skip_group_check=True)
                        nc.tensor.matmul(
                            pc[64:128, :], v_sb[:, t, gB, 0:64], et[1][:, t, :],
                            start=(t == 0), stop=(t == NT - 1),
                            tile_position=(0, 64), skip_group_check=True)
                    # softmax denominators: fp16 tree-sum over the 16 n-tiles,
                    # then one ones-matmul per head reduces over partitions
                    sums = []
                    for hh in range(2):
                        st = cpool.tile([128, NT // 2, MC], f16,
                                        tag=f"st{hh}", name=f"st{hh}",
                                        bufs=1)
                        nc.vector.tensor_add(
                            st, et[hh][:, 0:8, :], et[hh][:, 8:16, :])
                        nc.vector.tensor_add(
                            st[:, 0:4, :], st[:, 0:4, :], st[:, 4:8, :])
                        nc.vector.tensor_add(
                            st[:, 0:2, :], st[:, 0:2, :], st[:, 2:4, :])
                        sm = cpool.tile([128, MC], f16, tag=f"sum{hh}",
                                        name=f"sum{hh}")
                        nc.vector.tensor_add(sm, st[:, 0, :], st[:, 1, :])
                        sums.append(sm)
                    dn = ppsum.tile([33, MC], f32, tag="pp", name="dn")
                    nc.tensor.matmul(dn[0:1, :], onesk_sb, sums[0],
                                     start=True, stop=True, tile_position=(0, 0))
                    nc.tensor.matmul(dn[32:33, :], onesk_sb, sums[1],
                                     start=True, stop=True, tile_position=(0, 32))
                    ctx_sb = cpool.tile([128, MC], f16, tag="csb", name="csbp")
                    nc.vector.tensor_copy(out=ctx_sb, in_=pc)
                    den_sb = cpool.tile([33, MC], f32, tag="dsb", name="dsb")
                    nc.vector.tensor_copy(out=den_sb[0:1, :], in_=dn[0:1, :])
                    nc.vector.tensor_copy(out=den_sb[32:33, :],
                                          in_=dn[32:33, :])
                    out_sb = opool.tile([128, NMC, 128], f32, tag="osb")
                    tr = tpsum.tile([128, NMC, 128], f16, tag="tr", name="trp")
                    trd = ppsum.tile([128, NMC, 33], f32, tag="pp",
                                     name="trd")
                    for mt in range(NMC):
                        nc.tensor.transpose(
                            tr[:, mt, :], ctx_sb[:, mt * 128:(mt + 1) * 128],
                            identh)
                        nc.tensor.transpose(
                            trd[:, mt, :],
                            den_sb[:, mt * 128:(mt + 1) * 128],
                            ident[0:33, 0:33])
                    for mt in range(NMC):
                        rc = cpool.tile([128, 2], f32, tag="rc")
                        nc.vector.reciprocal(rc, trd[:, mt, 0:33:32])
                        for hh in range(2):
                            nc.vector.tensor_scalar_mul(
                                out_sb[:, mt, hh * 64:(hh + 1) * 64],
                                tr[:, mt, hh * 64:(hh + 1) * 64],
                                rc[:, hh:hh + 1])
                    for mt in range(NMC):
                        nc.sync.dma_start(
                            out=out[m * MC + mt * 128:m * MC + (mt + 1) * 128,
                                    j * 128:(j + 1) * 128],
                            in_=out_sb[:, mt, :])

                def ctxpart(j, m, et):
                    if paired:
                        return ctxpart_paired(j, m, et)
                    out_sb = opool.tile([128, NMC, 128], f32, tag="osb")
                    for hh in range(2):
                        g = 2 * j + hh
                        pc = cpsum.tile([65, MC], f32, tag="ctx")
                        for t in range(NT):
                            nc.tensor.matmul(
                                pc, v_sb[:, t, g, :], et[hh][:, t, :],
                                start=(t == 0), stop=(t == NT - 1))
                        ctx_sb = cpool.tile([65, MC], f32, tag="csb")
                        nc.vector.tensor_copy(out=ctx_sb, in_=pc)
                        tr = tpsum.tile([128, NMC, 65], f32, tag="tr")
                        for mt in range(NMC):
                            nc.tensor.transpose(
                                tr[:, mt, :],
                                ctx_sb[:, mt * 128:(mt + 1) * 128], ident)
                        for mt in range(NMC):
                            rc = cpool.tile([128, 1], f32, tag="rc")
                            nc.vector.reciprocal(rc, tr[:, mt, 64:65])
                            nc.vector.tensor_scalar_mul(
                                out_sb[:, mt, hh * 64:(hh + 1) * 64],
                                tr[:, mt, 0:64], rc)
                    for mt in range(NMC):
                        nc.sync.dma_start(
                            out=out[m * MC + mt * 128:m * MC + (mt + 1) * 128,
                                    j * 128:(j + 1) * 128],
                            in_=out_sb[:, mt, :])

                # Software-pipelined emission: scores of unit u+1 are emitted
                # before ctx of unit u so ACT (exp) always has PE-fed work.
                units = [(j, m) for m in range(NMC) for j in range(HPC // 2)]
                pending = None       # (j, m, et) awaiting ctxpart
                for u, (j, m) in enumerate(units):
                    if m == 0:
                        kproj(j)
                    qproj(j, m)
                    et = scores(j, m)
                    if u == 0:
                        vproj()      # overlaps with exp of unit 0 on ACT
                    if pending is not None:
                        ctxpart(*pending)
                    pending = (j, m, et)
                ctxpart(*pending)
            if reps != 1:
                rep_stack.close()

    nc.finalize()
    return nc


def _get_nc(has_bv: bool, reps: int = 1, paired: bool = False):
    key = ("nc", has_bv, reps, paired)
    if key not in _CACHE:
        _CACHE[key] = _build(has_bv, reps, paired)
    return _CACHE[key]


def _prep_in_maps(hidden_states, attention_mask, Wq, bq, Wk, bk, Wv, bv):
    hs = np.ascontiguousarray(np.asarray(hidden_states, dtype=np.float32))
    mask = np.asarray(attention_mask, dtype=np.float32)
    Wq = np.asarray(Wq, dtype=np.float32)
    Wk = np.asarray(Wk, dtype=np.float32)
    Wv = np.asarray(Wv, dtype=np.float32)
    bq = np.asarray(bq, dtype=np.float32)
    bk = np.asarray(bk, dtype=np.float32)
    bv = np.asarray(bv, dtype=np.float32)
    scale = 1.0 / np.sqrt(np.float32(HD))
    has_bv = bool(np.any(bv != 0.0))

    in_maps = []
    for c in range(8):
        b, hg = c // 2, c % 2
        sl = slice(hg * OC, (hg + 1) * OC)
        m = {
            "xt": np.ascontiguousarray(hs[b].T.astype(np.float16)),
            "wqt": np.ascontiguousarray((Wq[sl] * scale).T.astype(np.float16)),
            "wkt": np.ascontiguousarray(Wk[sl].T.astype(np.float16)),
            "wvt": np.ascontiguousarray(Wv[sl].T.astype(np.float16)),
            "bqt": np.ascontiguousarray((bq[sl] * scale).reshape(OC // 128, 128).T),
            "bkt": np.ascontiguousarray(bk[sl].reshape(OC // 128, 128).T),
            "maskt": np.ascontiguousarray(np.exp(mask[b]).reshape(NT, 128).T),
        }
        if has_bv:
            m["bv"] = np.ascontiguousarray(bv[sl].reshape(1, OC).astype(np.float16))
        in_maps.append(m)
    return in_maps, has_bv


def kernel(hidden_states, attention_mask, Wq, bq, Wk, bk, Wv, bv):
    from concourse import bass_utils

    in_maps, has_bv = _prep_in_maps(
        hidden_states, attention_mask, Wq, bq, Wk, bk, Wv, bv)
    # the faster paired-context variant folds no mask weights into the
    # denominators, so it requires an all-zero additive mask
    paired = not bool(np.any(np.asarray(attention_mask, dtype=np.float32)))
    nc = _get_nc(has_bv, paired=paired)
    res = bass_utils.run_bass_kernel_spmd(nc, in_maps, core_ids=list(range(8)))
    full = np.empty((B, S, H), dtype=np.float32)
    for c in range(8):
        b, hg = c // 2, c % 2
        full[b, :, hg * OC:(hg + 1) * OC] = res.results[c]["out"]
    return full



# revision 20
# speedup vs baseline: 277.3877x; 277.3877x over previous
"""BertSelfAttention kernel for Trainium2 (Bass/Tile), 8-core SPMD.

Full inputs in, full output out. Sharding: core c handles batch b = c//2 and
head-group hg = c%2 (8 of the 16 heads). Each core computes q/k/v projections
for its 512 features and full attention for its 8 heads; the host assembles
out[b, :, hg*512:(hg+1)*512] from each core. No collectives.

Structure (per core): a single 128-step software pipeline; step n emits
  - 4 score matmuls (2 k-tiles x 2 heads of the current head-pair, f16,
    head-packed via tile_position)
  - 2 exp activations (ACT, psum->sbuf f16) for the pair of k-tiles
  - 4 context matmuls for step n-2 (f16, M=65: the 65th stationary column is
    exp(attention_mask), so the softmax denominator accumulates alongside)
  - a dripped projection matmul group (q/k/v, f16) feeding later steps
Context+denominator PSUM tiles are DMAed straight to DRAM; the host divides
by the denominator row and transposes. ACT runs ~266us of exp; PE ~300us of
matmul; everything else hides underneath.

Problem shapes (hardcoded): B=4, S=2048, H=1024, nh=16, hd=64.
"""

import numpy as np

B, S, H = 4, 2048, 1024
NH, HD = 16, 64
HPC = 8          # heads per core
OC = HPC * HD    # output features per core (512)
NT = S // 128    # key tiles (16)
MC = 512         # m chunk (q positions per unit)
NMC = S // MC    # 4
KC = H // 128    # contraction chunks for projections (8)
NJ = HPC // 2    # head pairs (4)
OROW = HPC * 65  # output rows: per head 64 ctx features + 1 denominator

_CACHE = {}


def _build(has_bv: bool):
    from contextlib import ExitStack

    import concourse.bass as bass
    from concourse import bacc
    import concourse.tile as tile
    from concourse import mybir

    f32 = mybir.dt.float32
    f16 = mybir.dt.float16

    nc = bacc.Bacc(trn_type="TRN2")

    xT = nc.dram_tensor("xt", [H, S], f16, kind="ExternalInput")
    wqT = nc.dram_tensor("wqt", [H, OC], f16, kind="ExternalInput")
    wkT = nc.dram_tensor("wkt", [H, OC], f16, kind="ExternalInput")
    wvT = nc.dram_tensor("wvt", [H, OC], f16, kind="ExternalInput")
    bqT = nc.dram_tensor("bqt", [128, OC // 128], f32, kind="ExternalInput")
    bkT = nc.dram_tensor("bkt", [128, OC // 128], f32, kind="ExternalInput")
    maskT = nc.dram_tensor("maskt", [128, NT], f32, kind="ExternalInput")
    if has_bv:
        bv = nc.dram_tensor("bv", [1, OC], f16, kind="ExternalInput")
    out = nc.dram_tensor("out", [OROW, S], f32, kind="ExternalOutput")

    xT_r = xT[:].rearrange("(c p) s -> p c s", p=128)      # [128, KC, S]
    wqT_r = wqT[:].rearrange("(c p) o -> p c o", p=128)    # [128, KC, OC]
    wkT_r = wkT[:].rearrange("(c p) o -> p c o", p=128)
    wvT_r = wvT[:].rearrange("(c p) o -> p c o", p=128)

    # j-major: kT[j] is first needed at unit 4j, so kproj(1..3) can drip
    # into the late steps where PE otherwise idles against the ACT pace.
    units = [(j, m) for j in range(NJ) for m in range(NMC)]

    with tile.TileContext(nc) as tc, ExitStack() as ctx:
        consts = ctx.enter_context(tc.tile_pool(name="consts", bufs=1))
        mask_sb = consts.tile([128, NT], f32)
        eshift_sb = consts.tile([128, 1], f32)
        nc.vector.memset(eshift_sb, -4.0)
        bq_sb = consts.tile([128, OC // 128], f32)
        bk_sb = consts.tile([128, OC // 128], f32)
        if has_bv:
            bv_sb = consts.tile([1, OC], f16)
            ones_sb = consts.tile([1, 128], f16)
            nc.vector.memset(ones_sb, 1.0)

        # Persistent activations
        qkv = ctx.enter_context(tc.tile_pool(name="qkv", bufs=1))
        qT_sb = qkv.tile([128, NJ, S], f16)          # [d-pair, j, pos]
        kT_sb = qkv.tile([128, NJ, S], f16)
        v_sb = qkv.tile([128, NT, HPC, 65], f16)     # [key, tile, head, d+den]

        xw = ctx.enter_context(tc.tile_pool(name="xw", bufs=1))
        wk_sb = xw.tile([128, KC, OC], f16)
        wq_sb = xw.tile([128, KC, OC], f16)
        wv_sb = xw.tile([128, KC, OC], f16)
        xs = [xw.tile([128, KC, MC], f16, name=f"xs{s}") for s in range(NMC)]

        # DMA prologue, in first-use order, gating pieces split small:
        # kproj(0,0)'s MM for contraction chunk i needs only wk[:, i, 0:128]
        # and xs[0][:, i, :], so interleave those pieces, smallest first.
        for lo, hi in ((0, 2), (2, 4), (4, 6), (6, 8)):
            nc.sync.dma_start(out=wk_sb[:, lo:hi, 0:128],
                              in_=wkT_r[:, lo:hi, 0:128])
            nc.sync.dma_start(out=xs[0][:, lo:hi, :],
                              in_=xT_r[:, lo:hi, 0:MC])
        nc.sync.dma_start(out=wq_sb[:, :, 0:128], in_=wqT_r[:, :, 0:128])
        nc.sync.dma_start(out=bk_sb, in_=bkT[:])
        nc.sync.dma_start(out=bq_sb, in_=bqT[:])
        nc.sync.dma_start(out=mask_sb, in_=maskT[:])
        for h in range(4):
            nc.sync.dma_start(out=wv_sb[:, 2 * h:2 * h + 2, :],
                              in_=wvT_r[:, 2 * h:2 * h + 2, :])
        for s in range(1, NMC):
            nc.sync.dma_start(out=xs[s], in_=xT_r[:, :, s * MC:(s + 1) * MC])
        nc.sync.dma_start(out=wk_sb[:, :, 128:OC], in_=wkT_r[:, :, 128:OC])
        nc.sync.dma_start(out=wq_sb[:, :, 128:OC], in_=wqT_r[:, :, 128:OC])
        if has_bv:
            nc.sync.dma_start(out=bv_sb, in_=bv[:])

        with tc.tile_pool(name="exp", bufs=3) as epool, \
             tc.tile_pool(name="csb", bufs=2) as cspool, \
             tc.tile_pool(name="pps", bufs=2, space="PSUM") as ppsum, \
             tc.tile_pool(name="sps", bufs=2, space="PSUM") as spsum, \
             tc.tile_pool(name="cps", bufs=1, space="PSUM") as cpsum:

            # denominator weights: exp(mask) column per head
            for gt in range(NT):
                nc.vector.tensor_copy(
                    out=v_sb[:, gt, :, 64:65],
                    in_=mask_sb[:, gt:gt + 1].to_broadcast([128, HPC, 1]))

            def kproj(j, s):
                ss = slice(s * MC, (s + 1) * MC)
                psk = ppsum.tile([128, MC], f32, tag="pp", name="psk")
                for i in range(KC):
                    nc.tensor.matmul(
                        psk, wk_sb[:, i, j * 128:(j + 1) * 128],
                        xs[s][:, i, :], start=(i == 0), stop=(i == KC - 1))
                nc.vector.tensor_scalar_add(
                    kT_sb[:, j, ss], psk, bk_sb[:, j:j + 1])

            def qproj(j, m):
                ms = slice(m * MC, (m + 1) * MC)
                psq = ppsum.tile([128, MC], f32, tag="pp", name="psq")
                for i in range(KC):
                    nc.tensor.matmul(
                        psq, wq_sb[:, i, j * 128:(j + 1) * 128],
                        xs[m][:, i, :], start=(i == 0), stop=(i == KC - 1))
                nc.vector.tensor_scalar_add(
                    qT_sb[:, j, ms], psq, bq_sb[:, j:j + 1])

            def vproj(gt):
                psv = ppsum.tile([128, OC], f32, tag="pp", name="psv")
                for i in range(KC):
                    nc.tensor.matmul(
                        psv, xs[gt // 4][:, i, (gt % 4) * 128:(gt % 4 + 1) * 128],
                        wv_sb[:, i, :], start=(i == 0),
                        stop=(i == KC - 1 and not has_bv))
                if has_bv:
                    nc.tensor.matmul(psv, ones_sb, bv_sb,
                                     start=False, stop=True)
                nc.vector.tensor_scalar_mul(
                    v_sb[:, gt, :, 0:64],
                    psv.rearrange("p (h d) -> p h d", h=HPC),
                    mask_sb[:, gt:gt + 1])

            # Projection drip queue ordered by due step (the step before
            # whose score/context matmuls need the result).  Scores at unit
            # u=4j+m, step i consume kT keys < 256(i+1) (kproj chunk s covers
            # keys [512s, 512s+512)) and qT[j, m]; ctx at step n+2 consumes
            # v tiles 2(n%8), 2(n%8)+1.
            tasks = []   # (due_step, fn, args)
            for j in range(NJ):
                for s in range(NMC):
                    if (j, s) != (0, 0):
                        tasks.append((8 * 4 * j + 2 * s - 1, kproj, (j, s)))
            for j in range(NJ):
                for m in range(NMC):
                    if (j, m) != (0, 0):
                        tasks.append((8 * (4 * j + m) - 1, qproj, (j, m)))
            for gt in range(NT):
                tasks.append((max(gt // 2 - 1, 0), vproj, (gt,)))
            # Backward-pack: place each task at the latest free step at or
            # before its deadline (1 task/step past the forced front), so the
            # drip fills the late steps where sc+ctx alone run below the ACT
            # pace and PE would otherwise idle.
            NSTEP = len(units) * 8              # 128
            slots = {}                          # step -> [task]
            for due, fn, args in sorted(tasks, key=lambda t: -t[0]):
                s = min(due, NSTEP - 1)
                while s > 9 and slots.get(s):
                    s -= 1
                slots.setdefault(s, []).append((fn, args))

            # prologue: just enough for unit (0,0) step 0
            kproj(0, 0)
            qproj(0, 0)

            et_ring = {}                        # step -> (et_h0, et_h1)
            pc_cur = [None, None]               # open ctx psum per head slot

            def ctx_group(n):
                u, i = divmod(n, 8)
                j, m = units[u]
                for hh in range(2):
                    if i == 0:
                        pc_cur[hh] = cpsum.tile([65, MC], f32,
                                                tag=f"ctx{hh}",
                                                name=f"pc{hh}")
                    pc = pc_cur[hh]
                    g = 2 * j + hh
                    et = et_ring[n][hh]
                    for tl in range(2):
                        t = 2 * i + tl
                        nc.tensor.matmul(
                            pc, v_sb[:, t, g, :], et[:, tl, :],
                            start=(t == 0), stop=(t == NT - 1),
                            skip_group_check=True)
                    if i == 7:
                        cs = cspool.tile([65, MC], f32, tag=f"cs{hh}",
                                         name=f"cs{hh}")
                        nc.vector.tensor_copy(out=cs, in_=pc)
                        nc.sync.dma_start(
                            out=out[g * 65:(g + 1) * 65, m * MC:(m + 1) * MC],
                            in_=cs)
                del et_ring[n]

            for step in range(NSTEP):
                u, i = divmod(step, 8)
                j, m = units[u]
                ms = slice(m * MC, (m + 1) * MC)
                # trailing context and projection drip go FIRST so PE does
                # useful work while the score psum slot waits on exp's ack
                if step >= 2:
                    ctx_group(step - 2)
                for fn, args in slots.get(step, ()):
                    fn(*args)
                # scores: 2 k-tiles x 2 heads
                sc = [spsum.tile([128, 2, MC], f32, tag="sc", name=f"sc{hh}")
                      for hh in range(2)]
                for tl in range(2):
                    t = 2 * i + tl
                    for hh in range(2):
                        nc.tensor.matmul(
                            sc[hh][:, tl, :],
                            kT_sb[hh * 64:(hh + 1) * 64, j,
                                  t * 128:(t + 1) * 128],
                            qT_sb[hh * 64:(hh + 1) * 64, j, ms],
                            start=True, stop=True,
                            tile_position=(hh * 64, 0))
                # exp (constant -4 shift cancels in softmax; guards ranges)
                ets = []
                for hh in range(2):
                    et = epool.tile([128, 2, MC], f16, tag=f"exp{hh}",
                                    name=f"et{hh}")
                    nc.scalar.activation(
                        out=et, in_=sc[hh],
                        func=mybir.ActivationFunctionType.Exp,
                        bias=eshift_sb[:, 0:1])
                    ets.append(et)
                et_ring[step] = ets
            ctx_group(NSTEP - 2)
            ctx_group(NSTEP - 1)

    nc.finalize()
    return nc


def _get_nc(has_bv: bool):
    key = ("nc", has_bv)
    if key not in _CACHE:
        _CACHE[key] = _build(has_bv)
    return _CACHE[key]


def _prep_in_maps(hidden_states, attention_mask, Wq, bq, Wk, bk, Wv, bv):
    hs = np.ascontiguousarray(np.asarray(hidden_states, dtype=np.float32))
    mask = np.asarray(attention_mask, dtype=np.float32)
    Wq = np.asarray(Wq, dtype=np.float32)
    Wk = np.asarray(Wk, dtype=np.float32)
    Wv = np.asarray(Wv, dtype=np.float32)
    bq = np.asarray(bq, dtype=np.float32)
    bk = np.asarray(bk, dtype=np.float32)
    bv = np.asarray(bv, dtype=np.float32)
    scale = 1.0 / np.sqrt(np.float32(HD))
    has_bv = bool(np.any(bv != 0.0))

    in_maps = []
    for c in range(8):
        b, hg = c // 2, c % 2
        sl = slice(hg * OC, (hg + 1) * OC)
        m = {
            "xt": np.ascontiguousarray(hs[b].T.astype(np.float16)),
            "wqt": np.ascontiguousarray((Wq[sl] * scale).T.astype(np.float16)),
            "wkt": np.ascontiguousarray(Wk[sl].T.astype(np.float16)),
            "wvt": np.ascontiguousarray(Wv[sl].T.astype(np.float16)),
            "bqt": np.ascontiguousarray((bq[sl] * scale).reshape(OC // 128, 128).T),
            "bkt": np.ascontiguousarray(bk[sl].reshape(OC // 128, 128).T),
            "maskt": np.ascontiguousarray(np.exp(mask[b]).reshape(NT, 128).T),
        }
        if has_bv:
            m["bv"] = np.ascontiguousarray(bv[sl].reshape(1, OC).astype(np.float16))
        in_maps.append(m)
    return in_maps, has_bv


def kernel(hidden_states, attention_mask, Wq, bq, Wk, bk, Wv, bv):
    from concourse import bass_utils

    in_maps, has_bv = _prep_in_maps(
        hidden_states, attention_mask, Wq, bq, Wk, bk, Wv, bv)
    nc = _get_nc(has_bv)
    res = bass_utils.run_bass_kernel_spmd(nc, in_maps, core_ids=list(range(8)))
    full = np.empty((B, S, H), dtype=np.float32)
    for c in range(8):
        b, hg = c // 2, c % 2
        o = res.results[c]["out"].reshape(HPC, 65, S)
        ctx = o[:, :64, :] / o[:, 64:65, :]             # softmax denominator
        full[b, :, hg * OC:(hg + 1) * OC] = \
            ctx.transpose(2, 0, 1).reshape(S, OC)
    return full


# revision 21
# speedup vs baseline: 278.2825x; 1.0032x over previous
"""BertSelfAttention kernel for Trainium2 (Bass/Tile), 8-core SPMD.

Full inputs in, full output out. Sharding: core c handles batch b = c//2 and
head-group hg = c%2 (8 of the 16 heads). Each core computes q/k/v projections
for its 512 features and full attention for its 8 heads; the host assembles
out[b, :, hg*512:(hg+1)*512] from each core. No collectives.

Structure (per core): a single 128-step software pipeline; step n emits
  - 4 score matmuls (2 k-tiles x 2 heads of the current head-pair, f16,
    head-packed via tile_position)
  - 2 exp activations (ACT, psum->sbuf f16) for the pair of k-tiles
  - 4 context matmuls for step n-2 (f16, M=65: the 65th stationary column is
    exp(attention_mask), so the softmax denominator accumulates alongside)
  - a dripped projection matmul group (q/k/v, f16) feeding later steps
Context+denominator PSUM tiles are DMAed straight to DRAM; the host divides
by the denominator row and transposes. ACT runs ~266us of exp; PE ~300us of
matmul; everything else hides underneath.

Problem shapes (hardcoded): B=4, S=2048, H=1024, nh=16, hd=64.
"""

import numpy as np

B, S, H = 4, 2048, 1024
NH, HD = 16, 64
HPC = 8          # heads per core
OC = HPC * HD    # output features per core (512)
NT = S // 128    # key tiles (16)
MC = 512         # m chunk (q positions per unit)
NMC = S // MC    # 4
KC = H // 128    # contraction chunks for projections (8)
NJ = HPC // 2    # head pairs (4)
OROW = HPC * 65  # output rows: per head 64 ctx features + 1 denominator

_CACHE = {}


def _build(has_bv: bool):
    from contextlib import ExitStack

    import concourse.bass as bass
    from concourse import bacc
    import concourse.tile as tile
    from concourse import mybir

    f32 = mybir.dt.float32
    f16 = mybir.dt.float16

    nc = bacc.Bacc(trn_type="TRN2")

    xT = nc.dram_tensor("xt", [H, S], f16, kind="ExternalInput")
    wqT = nc.dram_tensor("wqt", [H, OC], f16, kind="ExternalInput")
    wkT = nc.dram_tensor("wkt", [H, OC], f16, kind="ExternalInput")
    wvT = nc.dram_tensor("wvt", [H, OC], f16, kind="ExternalInput")
    # packed [bq(4) | bk(4) | exp(mask)(16)] to keep the DMA prologue short
    bqkmT = nc.dram_tensor("bqkm", [128, 2 * (OC // 128) + NT], f32,
                           kind="ExternalInput")
    if has_bv:
        bv = nc.dram_tensor("bv", [1, OC], f16, kind="ExternalInput")
    out = nc.dram_tensor("out", [OROW, S], f32, kind="ExternalOutput")

    xT_r = xT[:].rearrange("(c p) s -> p c s", p=128)      # [128, KC, S]
    wqT_r = wqT[:].rearrange("(c p) o -> p c o", p=128)    # [128, KC, OC]
    wkT_r = wkT[:].rearrange("(c p) o -> p c o", p=128)
    wvT_r = wvT[:].rearrange("(c p) o -> p c o", p=128)

    # j-major: kT[j] is first needed at unit 4j, so kproj(1..3) can drip
    # into the late steps where PE otherwise idles against the ACT pace.
    units = [(j, m) for j in range(NJ) for m in range(NMC)]

    with tile.TileContext(nc) as tc, ExitStack() as ctx:
        consts = ctx.enter_context(tc.tile_pool(name="consts", bufs=1))
        mask_sb = consts.tile([128, NT], f32)
        eshift_sb = consts.tile([128, 1], f32)
        nc.vector.memset(eshift_sb, -4.0)
        bq_sb = consts.tile([128, OC // 128], f32)
        bk_sb = consts.tile([128, OC // 128], f32)
        if has_bv:
            bv_sb = consts.tile([1, OC], f16)
            ones_sb = consts.tile([1, 128], f16)
            nc.vector.memset(ones_sb, 1.0)

        # Persistent activations
        qkv = ctx.enter_context(tc.tile_pool(name="qkv", bufs=1))
        qT_sb = qkv.tile([128, NJ, S], f16)          # [d-pair, j, pos]
        kT_sb = qkv.tile([128, NJ, S], f16)
        v_sb = qkv.tile([128, NT, HPC, 65], f16)     # [key, tile, head, d+den]

        xw = ctx.enter_context(tc.tile_pool(name="xw", bufs=1))
        wk_sb = xw.tile([128, KC, OC], f16)
        wq_sb = xw.tile([128, KC, OC], f16)
        wv_sb = xw.tile([128, KC, OC], f16)
        xs = [xw.tile([128, KC, MC], f16, name=f"xs{s}") for s in range(NMC)]

        # DMA prologue, in first-use order, gating pieces split small:
        # kproj(0,0)'s MM for contraction chunk i needs only wk[:, i, 0:128]
        # and xs[0][:, i, :], so interleave those pieces, smallest first.
        for lo, hi in ((0, 2), (2, 4), (4, 6), (6, 8)):
            nc.sync.dma_start(out=wk_sb[:, lo:hi, 0:128],
                              in_=wkT_r[:, lo:hi, 0:128])
            nc.sync.dma_start(out=xs[0][:, lo:hi, :],
                              in_=xT_r[:, lo:hi, 0:MC])
        nc.sync.dma_start(out=wq_sb[:, :, 0:128], in_=wqT_r[:, :, 0:128])
        nc.sync.dma_start(out=bk_sb, in_=bkT[:])
        nc.sync.dma_start(out=bq_sb, in_=bqT[:])
        nc.sync.dma_start(out=mask_sb, in_=maskT[:])
        for h in range(4):
            nc.sync.dma_start(out=wv_sb[:, 2 * h:2 * h + 2, :],
                              in_=wvT_r[:, 2 * h:2 * h + 2, :])
        for s in range(1, NMC):
            nc.sync.dma_start(out=xs[s], in_=xT_r[:, :, s * MC:(s + 1) * MC])
        nc.sync.dma_start(out=wk_sb[:, :, 128:OC], in_=wkT_r[:, :, 128:OC])
        nc.sync.dma_start(out=wq_sb[:, :, 128:OC], in_=wqT_r[:, :, 128:OC])
        if has_bv:
            nc.sync.dma_start(out=bv_sb, in_=bv[:])

        with tc.tile_pool(name="exp", bufs=3) as epool, \
             tc.tile_pool(name="csb", bufs=2) as cspool, \
             tc.tile_pool(name="pps", bufs=2, space="PSUM") as ppsum, \
             tc.tile_pool(name="sps", bufs=2, space="PSUM") as spsum, \
             tc.tile_pool(name="cps", bufs=1, space="PSUM") as cpsum:

            # denominator weights: exp(mask) column per head
            for gt in range(NT):
                nc.vector.tensor_copy(
                    out=v_sb[:, gt, :, 64:65],
                    in_=mask_sb[:, gt:gt + 1].to_broadcast([128, HPC, 1]))

            def kproj(j, s):
                ss = slice(s * MC, (s + 1) * MC)
                psk = ppsum.tile([128, MC], f32, tag="pp", name="psk")
                for i in range(KC):
                    nc.tensor.matmul(
                        psk, wk_sb[:, i, j * 128:(j + 1) * 128],
                        xs[s][:, i, :], start=(i == 0), stop=(i == KC - 1))
                nc.vector.tensor_scalar_add(
                    kT_sb[:, j, ss], psk, bk_sb[:, j:j + 1])

            def qproj(j, m):
                ms = slice(m * MC, (m + 1) * MC)
                psq = ppsum.tile([128, MC], f32, tag="pp", name="psq")
                for i in range(KC):
                    nc.tensor.matmul(
                        psq, wq_sb[:, i, j * 128:(j + 1) * 128],
                        xs[m][:, i, :], start=(i == 0), stop=(i == KC - 1))
                nc.vector.tensor_scalar_add(
                    qT_sb[:, j, ms], psq, bq_sb[:, j:j + 1])

            def vproj(gt):
                psv = ppsum.tile([128, OC], f32, tag="pp", name="psv")
                for i in range(KC):
                    nc.tensor.matmul(
                        psv, xs[gt // 4][:, i, (gt % 4) * 128:(gt % 4 + 1) * 128],
                        wv_sb[:, i, :], start=(i == 0),
                        stop=(i == KC - 1 and not has_bv))
                if has_bv:
                    nc.tensor.matmul(psv, ones_sb, bv_sb,
                                     start=False, stop=True)
                nc.vector.tensor_scalar_mul(
                    v_sb[:, gt, :, 0:64],
                    psv.rearrange("p (h d) -> p h d", h=HPC),
                    mask_sb[:, gt:gt + 1])

            # Projection drip queue ordered by due step (the step before
            # whose score/context matmuls need the result).  Scores at unit
            # u=4j+m, step i consume kT keys < 256(i+1) (kproj chunk s covers
            # keys [512s, 512s+512)) and qT[j, m]; ctx at step n+2 consumes
            # v tiles 2(n%8), 2(n%8)+1.
            tasks = []   # (due_step, fn, args)
            for j in range(NJ):
                for s in range(NMC):
                    if (j, s) != (0, 0):
                        tasks.append((8 * 4 * j + 2 * s - 1, kproj, (j, s)))
            for j in range(NJ):
                for m in range(NMC):
                    if (j, m) != (0, 0):
                        tasks.append((8 * (4 * j + m) - 1, qproj, (j, m)))
            for gt in range(NT):
                tasks.append((max(gt // 2 - 1, 0), vproj, (gt,)))
            # Backward-pack: place each task at the latest free step at or
            # before its deadline (1 task/step past the forced front), so the
            # drip fills the late steps where sc+ctx alone run below the ACT
            # pace and PE would otherwise idle.
            NSTEP = len(units) * 8              # 128
            slots = {}                          # step -> [task]
            for due, fn, args in sorted(tasks, key=lambda t: -t[0]):
                s = min(due, NSTEP - 1)
                while s > 9 and slots.get(s):
                    s -= 1
                slots.setdefault(s, []).append((fn, args))

            # prologue: just enough for unit (0,0) step 0
            kproj(0, 0)
            qproj(0, 0)

            et_ring = {}                        # step -> (et_h0, et_h1)
            pc_cur = [None, None]               # open ctx psum per head slot

            def ctx_group(n):
                u, i = divmod(n, 8)
                j, m = units[u]
                for hh in range(2):
                    if i == 0:
                        pc_cur[hh] = cpsum.tile([65, MC], f32,
                                                tag=f"ctx{hh}",
                                                name=f"pc{hh}")
                    pc = pc_cur[hh]
                    g = 2 * j + hh
                    et = et_ring[n][hh]
                    for tl in range(2):
                        t = 2 * i + tl
                        nc.tensor.matmul(
                            pc, v_sb[:, t, g, :], et[:, tl, :],
                            start=(t == 0), stop=(t == NT - 1),
                            skip_group_check=True)
                    if i == 7:
                        cs = cspool.tile([65, MC], f32, tag=f"cs{hh}",
                                         name=f"cs{hh}")
                        nc.vector.tensor_copy(out=cs, in_=pc)
                        nc.sync.dma_start(
                            out=out[g * 65:(g + 1) * 65, m * MC:(m + 1) * MC],
                            in_=cs)
                del et_ring[n]

            for step in range(NSTEP):
                u, i = divmod(step, 8)
                j, m = units[u]
                ms = slice(m * MC, (m + 1) * MC)
                # trailing context and projection drip go FIRST so PE does
                # useful work while the score psum slot waits on exp's ack
                if step >= 2:
                    ctx_group(step - 2)
                for fn, args in slots.get(step, ()):
                    fn(*args)
                # scores: 2 k-tiles x 2 heads
                sc = [spsum.tile([128, 2, MC], f32, tag="sc", name=f"sc{hh}")
                      for hh in range(2)]
                for tl in range(2):
                    t = 2 * i + tl
                    for hh in range(2):
                        nc.tensor.matmul(
                            sc[hh][:, tl, :],
                            kT_sb[hh * 64:(hh + 1) * 64, j,
                                  t * 128:(t + 1) * 128],
                            qT_sb[hh * 64:(hh + 1) * 64, j, ms],
                            start=True, stop=True,
                            tile_position=(hh * 64, 0))
                # exp (constant -4 shift cancels in softmax; guards ranges)
                ets = []
                for hh in range(2):
                    et = epool.tile([128, 2, MC], f16, tag=f"exp{hh}",
                                    name=f"et{hh}")
                    nc.scalar.activation(
                        out=et, in_=sc[hh],
                        func=mybir.ActivationFunctionType.Exp,
                        bias=eshift_sb[:, 0:1])
                    ets.append(et)
                et_ring[step] = ets
            ctx_group(NSTEP - 2)
            ctx_group(NSTEP - 1)

    nc.finalize()
    return nc


def _get_nc(has_bv: bool):
    key = ("nc", has_bv)
    if key not in _CACHE:
        _CACHE[key] = _build(has_bv)
    return _CACHE[key]


def _prep_in_maps(hidden_states, attention_mask, Wq, bq, Wk, bk, Wv, bv):
    hs = np.ascontiguousarray(np.asarray(hidden_states, dtype=np.float32))
    mask = np.asarray(attention_mask, dtype=np.float32)
    Wq = np.asarray(Wq, dtype=np.float32)
    Wk = np.asarray(Wk, dtype=np.float32)
    Wv = np.asarray(Wv, dtype=np.float32)
    bq = np.asarray(bq, dtype=np.float32)
    bk = np.asarray(bk, dtype=np.float32)
    bv = np.asarray(bv, dtype=np.float32)
    scale = 1.0 / np.sqrt(np.float32(HD))
    has_bv = bool(np.any(bv != 0.0))

    in_maps = []
    for c in range(8):
        b, hg = c // 2, c % 2
        sl = slice(hg * OC, (hg + 1) * OC)
        m = {
            "xt": np.ascontiguousarray(hs[b].T.astype(np.float16)),
            "wqt": np.ascontiguousarray((Wq[sl] * scale).T.astype(np.float16)),
            "wkt": np.ascontiguousarray(Wk[sl].T.astype(np.float16)),
            "wvt": np.ascontiguousarray(Wv[sl].T.astype(np.float16)),
            "bqt": np.ascontiguousarray((bq[sl] * scale).reshape(OC // 128, 128).T),
            "bkt": np.ascontiguousarray(bk[sl].reshape(OC // 128, 128).T),
            "maskt": np.ascontiguousarray(np.exp(mask[b]).reshape(NT, 128).T),
        }
        if has_bv:
            m["bv"] = np.ascontiguousarray(bv[sl].reshape(1, OC).astype(np.float16))
        in_maps.append(m)
    return in_maps, has_bv


def kernel(hidden_states, attention_mask, Wq, bq, Wk, bk, Wv, bv):
    from concourse import bass_utils

    in_maps, has_bv = _prep_in_maps(
        hidden_states, attention_mask, Wq, bq, Wk, bk, Wv, bv)
    nc = _get_nc(has_bv)
    res = bass_utils.run_bass_kernel_spmd(nc, in_maps, core_ids=list(range(8)))
    full = np.empty((B, S, H), dtype=np.float32)
    for c in range(8):
        b, hg = c // 2, c % 2
        o = res.results[c]["out"].reshape(HPC, 65, S)
        ctx = o[:, :64, :] / o[:, 64:65, :]             # softmax denominator
        full[b, :, hg * OC:(hg + 1) * OC] = \
            ctx.transpose(2, 0, 1).reshape(S, OC)
    return full


# revision 36
# speedup vs baseline: 279.9945x; 1.0062x over previous
"""BertSelfAttention kernel for Trainium2 (Bass/Tile), 8-core SPMD.

Full inputs in, full output out. Sharding: core c handles batch b = c//2 and
head-group hg = c%2 (8 of the 16 heads). Each core computes q/k/v projections
for its 512 features and full attention for its 8 heads; the host assembles
out[b, :, hg*512:(hg+1)*512] from each core. No collectives.

Structure (per core): a single 128-step software pipeline; step n emits
  - 4 score matmuls (2 k-tiles x 2 heads of the current head-pair, f16,
    head-packed via tile_position)
  - 2 exp activations (ACT, psum->sbuf f16) for the pair of k-tiles
  - 4 context matmuls for step n-2 (f16, M=65: the 65th stationary column is
    exp(attention_mask), so the softmax denominator accumulates alongside)
  - a dripped projection matmul group (q/k/v, f16) feeding later steps
Context+denominator PSUM tiles are DMAed straight to DRAM; the host divides
by the denominator row and transposes. ACT runs ~266us of exp; PE ~300us of
matmul; everything else hides underneath.

Problem shapes (hardcoded): B=4, S=2048, H=1024, nh=16, hd=64.
"""

import numpy as np

B, S, H = 4, 2048, 1024
NH, HD = 16, 64
HPC = 8          # heads per core
OC = HPC * HD    # output features per core (512)
NT = S // 128    # key tiles (16)
MC = 512         # m chunk (q positions per unit)
NMC = S // MC    # 4
KC = H // 128    # contraction chunks for projections (8)
NJ = HPC // 2    # head pairs (4)
OROW = HPC * 65  # output rows: per head 64 ctx features + 1 denominator

_CACHE = {}


def _build(has_bv: bool):
    from contextlib import ExitStack

    import concourse.bass as bass
    from concourse import bacc
    import concourse.tile as tile
    from concourse import mybir

    f32 = mybir.dt.float32
    f16 = mybir.dt.float16

    nc = bacc.Bacc(trn_type="TRN2")

    xT = nc.dram_tensor("xt", [H, S], f16, kind="ExternalInput")
    # [128, NJ, KC, 128]: per-head-pair column blocks contiguous per
    # partition row, so block DMAs move 2KB descriptors (full bandwidth)
    wqT = nc.dram_tensor("wqt", [128, NJ * KC * 128], f16,
                         kind="ExternalInput")
    wkT = nc.dram_tensor("wkt", [128, NJ * KC * 128], f16,
                         kind="ExternalInput")
    wvT = nc.dram_tensor("wvt", [H, OC], f16, kind="ExternalInput")
    # packed [bq(4) | bk(4) | exp(mask)(16)] to keep the DMA prologue short
    bqkmT = nc.dram_tensor("bqkm", [128, 2 * (OC // 128) + NT], f32,
                           kind="ExternalInput")
    if has_bv:
        bv = nc.dram_tensor("bv", [1, OC], f16, kind="ExternalInput")
    out = nc.dram_tensor("out", [OROW, S], f32, kind="ExternalOutput")

    xT_r = xT[:].rearrange("(c p) s -> p c s", p=128)      # [128, KC, S]
    wqT_r = wqT[:].rearrange("p (j c f) -> p j c f", j=NJ, c=KC)
    wkT_r = wkT[:].rearrange("p (j c f) -> p j c f", j=NJ, c=KC)
    wvT_r = wvT[:].rearrange("(c p) o -> p c o", p=128)

    # j-major: kT[j] is first needed at unit 4j, so kproj(1..3) can drip
    # into the late steps where PE otherwise idles against the ACT pace.
    units = [(j, m) for j in range(NJ) for m in range(NMC)]

    with tile.TileContext(nc) as tc, ExitStack() as ctx:
        consts = ctx.enter_context(tc.tile_pool(name="consts", bufs=1))
        bqkm_sb = consts.tile([128, 2 * (OC // 128) + NT], f32)
        bq_sb = bqkm_sb[:, 0:4]
        bk_sb = bqkm_sb[:, 4:8]
        mask_sb = bqkm_sb[:, 8:8 + NT]
        eshift_sb = consts.tile([128, 1], f32)
        nc.vector.memset(eshift_sb, -4.0)
        if has_bv:
            bv_sb = consts.tile([1, OC], f16)
            ones_sb = consts.tile([1, 128], f16)
            nc.vector.memset(ones_sb, 1.0)

        # Persistent activations
        qkv = ctx.enter_context(tc.tile_pool(name="qkv", bufs=1))
        qT_sb = qkv.tile([128, NJ, S], f16)          # [d-pair, j, pos]
        kT_sb = qkv.tile([128, NJ, S], f16)
        v_sb = qkv.tile([128, NT, HPC, 65], f16)     # [key, tile, head, d+den]

        xw = ctx.enter_context(tc.tile_pool(name="xw", bufs=1))
        wk_sb = xw.tile([128, NJ, KC, 128], f16)
        wq_sb = xw.tile([128, NJ, KC, 128], f16)
        wv_sb = xw.tile([128, KC, OC], f16)
        xs = [xw.tile([128, KC, MC], f16, name=f"xs{s}") for s in range(NMC)]

        # DMA prologue, in first-use order, gating pieces split small:
        # kproj(0,0)'s MM for contraction chunk i needs only wk[:, i, 0:128]
        # and xs[0][:, i, :], so interleave those pieces, smallest first.
        for lo, hi in ((0, 2), (2, 4), (4, 6), (6, 8)):
            nc.sync.dma_start(out=wk_sb[:, 0, lo:hi, :],
                              in_=wkT_r[:, 0, lo:hi, :])
            nc.sync.dma_start(out=xs[0][:, lo:hi, :],
                              in_=xT_r[:, lo:hi, 0:MC])
        nc.sync.dma_start(out=wq_sb[:, 0], in_=wqT_r[:, 0])
        nc.sync.dma_start(out=bqkm_sb, in_=bqkmT[:])
        nc.sync.dma_start(out=xs[1], in_=xT_r[:, :, MC:2 * MC])
        for h in range(4):
            nc.sync.dma_start(out=wv_sb[:, 2 * h:2 * h + 2, :],
                              in_=wvT_r[:, 2 * h:2 * h + 2, :])
        for s in range(2, NMC):
            nc.sync.dma_start(out=xs[s], in_=xT_r[:, :, s * MC:(s + 1) * MC])
        nc.sync.dma_start(out=wk_sb[:, 1:NJ], in_=wkT_r[:, 1:NJ])
        nc.sync.dma_start(out=wq_sb[:, 1:NJ], in_=wqT_r[:, 1:NJ])
        if has_bv:
            nc.sync.dma_start(out=bv_sb, in_=bv[:])

        with tc.tile_pool(name="exp", bufs=7) as epool, \
             tc.tile_pool(name="csb", bufs=2) as cspool, \
             tc.tile_pool(name="pps", bufs=2, space="PSUM") as ppsum, \
             tc.tile_pool(name="sps", bufs=2, space="PSUM") as spsum, \
             tc.tile_pool(name="cps", bufs=1, space="PSUM") as cpsum:

            # denominator weights: exp(mask) column per head
            for gt in range(NT):
                nc.vector.tensor_copy(
                    out=v_sb[:, gt, :, 64:65],
                    in_=mask_sb[:, gt:gt + 1].to_broadcast([128, HPC, 1]))

            # Projections are emitted in two 4-matmul halves around each
            # step's score matmuls so PE load inside a step stays level.
            def kproj(j, s, half, cell):
                lo, hi = (0, KC // 2) if half == 0 else (KC // 2, KC)
                if half == 0:
                    cell["ps"] = ppsum.tile([128, MC], f32, tag="pp",
                                            name="psk")
                psk = cell["ps"]
                for i in range(lo, hi):
                    nc.tensor.matmul(
                        psk, wk_sb[:, j, i, :],
                        xs[s][:, i, :], start=(i == 0), stop=(i == KC - 1),
                        skip_group_check=True)
                if half == 1:
                    nc.vector.tensor_scalar_add(
                        kT_sb[:, j, s * MC:(s + 1) * MC], psk,
                        bk_sb[:, j:j + 1])

            def qproj(j, m, half, cell):
                lo, hi = (0, KC // 2) if half == 0 else (KC // 2, KC)
                if half == 0:
                    cell["ps"] = ppsum.tile([128, MC], f32, tag="pp",
                                            name="psq")
                psq = cell["ps"]
                for i in range(lo, hi):
                    nc.tensor.matmul(
                        psq, wq_sb[:, j, i, :],
                        xs[m][:, i, :], start=(i == 0), stop=(i == KC - 1),
                        skip_group_check=True)
                if half == 1:
                    nc.vector.tensor_scalar_add(
                        qT_sb[:, j, m * MC:(m + 1) * MC], psq,
                        bq_sb[:, j:j + 1])

            def vproj(gt, half, cell):
                lo, hi = (0, KC // 2) if half == 0 else (KC // 2, KC)
                if half == 0:
                    cell["ps"] = ppsum.tile([128, OC], f32, tag="pp",
                                            name="psv")
                psv = cell["ps"]
                for i in range(lo, hi):
                    nc.tensor.matmul(
                        psv, xs[gt // 4][:, i, (gt % 4) * 128:(gt % 4 + 1) * 128],
                        wv_sb[:, i, :], start=(i == 0),
                        stop=(i == KC - 1 and not has_bv),
                        skip_group_check=True)
                if half == 1:
                    if has_bv:
                        nc.tensor.matmul(psv, ones_sb, bv_sb,
                                         start=False, stop=True,
                                         skip_group_check=True)
                    nc.vector.tensor_scalar_mul(
                        v_sb[:, gt, :, 0:64],
                        psv.rearrange("p (h d) -> p h d", h=HPC),
                        mask_sb[:, gt:gt + 1])

            # Projection drip queue ordered by due step (the step before
            # whose score/context matmuls need the result).  Scores at unit
            # u=4j+m, step i consume kT keys < 256(i+1) (kproj chunk s covers
            # keys [512s, 512s+512)) and qT[j, m]; ctx at step n+2 consumes
            # v tiles 2(n%8), 2(n%8)+1.
            tasks = []   # (due_step, fn, args)
            for j in range(NJ):
                for s in range(NMC):
                    if (j, s) != (0, 0):
                        tasks.append((8 * 4 * j + 2 * s - 1, kproj, (j, s)))
            for j in range(NJ):
                for m in range(NMC):
                    if (j, m) != (0, 0):
                        tasks.append((8 * (4 * j + m) - 1, qproj, (j, m)))
            for gt in range(NT):
                tasks.append((max(gt // 2 + 1, 0), vproj, (gt,)))
            # Backward-pack with spacing: place each task at the latest free
            # step at or before its deadline, at most one per step past the
            # forced front and >=2 steps apart, so the drip fills the late
            # steps where sc+ctx alone run below the ACT pace.
            NSTEP = len(units) * 8              # 128
            slots = {}                          # step -> [task]
            for due, fn, args in sorted(tasks, key=lambda t: -t[0]):
                s = min(due, NSTEP - 1)
                while s > 9 and slots.get(s):
                    s -= 1
                slots.setdefault(s, []).append((fn, args))

            # prologue: just enough for unit (0,0) step 0
            c = {}
            kproj(0, 0, 0, c)
            kproj(0, 0, 1, c)
            c = {}
            qproj(0, 0, 0, c)
            qproj(0, 0, 1, c)

            et_ring = {}                        # step -> (et_h0, et_h1)
            pc_cur = [None, None]               # open ctx psum per head slot

            def ctx_group(n):
                u, i = divmod(n, 8)
                j, m = units[u]
                for hh in range(2):
                    if i == 0:
                        pc_cur[hh] = cpsum.tile([65, MC], f32,
                                                tag=f"ctx{hh}",
                                                name=f"pc{hh}")
                    pc = pc_cur[hh]
                    g = 2 * j + hh
                    et = et_ring[n][hh]
                    for tl in range(2):
                        t = 2 * i + tl
                        nc.tensor.matmul(
                            pc, v_sb[:, t, g, :], et[:, tl, :],
                            start=(t == 0), stop=(t == NT - 1),
                            skip_group_check=True)
                    if i == 7:
                        cs = cspool.tile([65, MC], f32, tag=f"cs{hh}",
                                         name=f"cs{hh}")
                        nc.vector.tensor_copy(out=cs, in_=pc)
                        nc.sync.dma_start(
                            out=out[g * 65:(g + 1) * 65, m * MC:(m + 1) * MC],
                            in_=cs)
                del et_ring[n]

            for step in range(NSTEP):
                u, i = divmod(step, 8)
                j, m = units[u]
                ms = slice(m * MC, (m + 1) * MC)
                # trailing context and drip half-0 go FIRST so PE does
                # useful work while the score psum slot waits on exp's ack;
                # drip half-1 lands after, leveling PE load within the step
                if step >= 4:
                    ctx_group(step - 4)
                pend = []
                if step >= 16:
                    # steady state: drip half-0 absorbs the sc ack-wait
                    for fn, args in slots.get(step, ()):
                        cell = {}
                        fn(*args, 0, cell)
                        pend.append((fn, args, cell))
                # scores: 2 k-tiles x 2 heads
                sc = [spsum.tile([128, 2, MC], f32, tag="sc", name=f"sc{hh}")
                      for hh in range(2)]
                for tl in range(2):
                    t = 2 * i + tl
                    for hh in range(2):
                        nc.tensor.matmul(
                            sc[hh][:, tl, :],
                            kT_sb[hh * 64:(hh + 1) * 64, j,
                                  t * 128:(t + 1) * 128],
                            qT_sb[hh * 64:(hh + 1) * 64, j, ms],
                            start=True, stop=True,
                            tile_position=(hh * 64, 0))
                # exp (constant -4 shift cancels in softmax; guards ranges)
                ets = []
                for hh in range(2):
                    et = epool.tile([128, 2, MC], f16, tag=f"exp{hh}",
                                    name=f"et{hh}")
                    nc.scalar.activation(
                        out=et, in_=sc[hh],
                        func=mybir.ActivationFunctionType.Exp,
                        bias=eshift_sb[:, 0:1])
                    ets.append(et)
                et_ring[step] = ets
                if step < 16:
                    # DMA-bound front: keep independent sc ahead of
                    # DMA-gated projection pieces in the PE FIFO
                    for fn, args in slots.get(step, ()):
                        cell = {}
                        fn(*args, 0, cell)
                        fn(*args, 1, cell)
                else:
                    for fn, args, cell in pend:
                        fn(*args, 1, cell)
            for n in range(NSTEP - 4, NSTEP):
                ctx_group(n)

    nc.finalize()
    return nc


def _get_nc(has_bv: bool):
    key = ("nc", has_bv)
    if key not in _CACHE:
        _CACHE[key] = _build(has_bv)
    return _CACHE[key]


def _prep_in_maps(hidden_states, attention_mask, Wq, bq, Wk, bk, Wv, bv):
    hs = np.ascontiguousarray(np.asarray(hidden_states, dtype=np.float32))
    mask = np.asarray(attention_mask, dtype=np.float32)
    Wq = np.asarray(Wq, dtype=np.float32)
    Wk = np.asarray(Wk, dtype=np.float32)
    Wv = np.asarray(Wv, dtype=np.float32)
    bq = np.asarray(bq, dtype=np.float32)
    bk = np.asarray(bk, dtype=np.float32)
    bv = np.asarray(bv, dtype=np.float32)
    scale = 1.0 / np.sqrt(np.float32(HD))
    has_bv = bool(np.any(bv != 0.0))

    in_maps = []
    for c in range(8):
        b, hg = c // 2, c % 2
        sl = slice(hg * OC, (hg + 1) * OC)
        bqkm = np.concatenate([
            (bq[sl] * scale).reshape(OC // 128, 128).T,
            bk[sl].reshape(OC // 128, 128).T,
            np.exp(mask[b]).reshape(NT, 128).T,
        ], axis=1).astype(np.float32)
        m = {
            "xt": np.ascontiguousarray(hs[b].T.astype(np.float16)),
            "wqt": np.ascontiguousarray(
                (Wq[sl] * scale).T.astype(np.float16).reshape(
                    KC, 128, NJ, 128).transpose(1, 2, 0, 3).reshape(
                    128, NJ * KC * 128)),
            "wkt": np.ascontiguousarray(
                Wk[sl].T.astype(np.float16).reshape(
                    KC, 128, NJ, 128).transpose(1, 2, 0, 3).reshape(
                    128, NJ * KC * 128)),
            "wvt": np.ascontiguousarray(Wv[sl].T.astype(np.float16)),
            "bqkm": np.ascontiguousarray(bqkm),
        }
        if has_bv:
            m["bv"] = np.ascontiguousarray(bv[sl].reshape(1, OC).astype(np.float16))
        in_maps.append(m)
    return in_maps, has_bv


def kernel(hidden_states, attention_mask, Wq, bq, Wk, bk, Wv, bv):
    from concourse import bass_utils

    in_maps, has_bv = _prep_in_maps(
        hidden_states, attention_mask, Wq, bq, Wk, bk, Wv, bv)
    nc = _get_nc(has_bv)
    res = bass_utils.run_bass_kernel_spmd(nc, in_maps, core_ids=list(range(8)))
    full = np.empty((B, S, H), dtype=np.float32)
    for c in range(8):
        b, hg = c // 2, c % 2
        o = res.results[c]["out"].reshape(HPC, 65, S)
        ctx = o[:, :64, :] / o[:, 64:65, :]             # softmax denominator
        full[b, :, hg * OC:(hg + 1) * OC] = \
            ctx.transpose(2, 0, 1).reshape(S, OC)
    return full


# revision 58
# speedup vs baseline: 282.7073x; 1.0097x over previous
"""BertSelfAttention kernel for Trainium2 (Bass/Tile), 8-core SPMD.

Full inputs in, full output out. Sharding: core c handles batch b = c//2 and
head-group hg = c%2 (8 of the 16 heads). Each core computes q/k/v projections
for its 512 features and full attention for its 8 heads; the host assembles
out[b, :, hg*512:(hg+1)*512] from each core. No collectives.

Structure (per core): a single 128-step software pipeline; step n emits
  - 4 score matmuls (2 k-tiles x 2 heads of the current head-pair, f16,
    head-packed via tile_position)
  - 2 exp activations (ACT, psum->sbuf f16, FD 1024) for the k-tile pair
  - 4 context matmuls trailing 4 steps (f16, M=65: the 65th stationary
    column is exp(attention_mask), so the softmax denominator accumulates
    alongside the context rows for free)
  - projection matmul half-groups (q/k/v, f16), backward-packed to the
    latest step allowed by their deadline; units iterate j-major so the
    k-projections of later head-pairs fill the late steps where scores +
    context alone run below the ACT exp pace
Context+denominator PSUM tiles are copied to SBUF and DMAed out
feature-major; the host divides by the denominator row and transposes.
PE runs ~300us of matmul (the bound), ACT ~266us of exp; DVE/DMA hide
underneath. TimelineSim: ~316us vs 372us for the previous kernel.

Problem shapes (hardcoded): B=4, S=2048, H=1024, nh=16, hd=64.
"""

import numpy as np

B, S, H = 4, 2048, 1024
NH, HD = 16, 64
HPC = 8          # heads per core
OC = HPC * HD    # output features per core (512)
NT = S // 128    # key tiles (16)
MC = 512         # m chunk (q positions per unit)
NMC = S // MC    # 4
KC = H // 128    # contraction chunks for projections (8)
NJ = HPC // 2    # head pairs (4)
OROW = HPC * 65  # output rows: per head 64 ctx features + 1 denominator

_CACHE = {}


def _build(has_bv: bool):
    from contextlib import ExitStack

    import concourse.bass as bass
    from concourse import bacc
    import concourse.tile as tile
    from concourse import mybir

    f32 = mybir.dt.float32
    f16 = mybir.dt.float16

    nc = bacc.Bacc(trn_type="TRN2")

    xT = nc.dram_tensor("xt", [H, S], f16, kind="ExternalInput")
    # [128, NJ, KC, 128]: per-head-pair column blocks contiguous per
    # partition row, so block DMAs move 2KB descriptors (full bandwidth)
    wqT = nc.dram_tensor("wqt", [128, NJ * KC * 128], f16,
                         kind="ExternalInput")
    wkT = nc.dram_tensor("wkt", [128, NJ * KC * 128], f16,
                         kind="ExternalInput")
    wvT = nc.dram_tensor("wvt", [H, OC], f16, kind="ExternalInput")
    # packed [bq(4) | bk(4) | exp(mask)(16)] to keep the DMA prologue short
    bqkmT = nc.dram_tensor("bqkm", [128, 2 * (OC // 128) + NT], f32,
                           kind="ExternalInput")
    if has_bv:
        bv = nc.dram_tensor("bv", [1, OC], f16, kind="ExternalInput")
    out = nc.dram_tensor("out", [OROW, S], f32, kind="ExternalOutput")

    xT_r = xT[:].rearrange("(c p) s -> p c s", p=128)      # [128, KC, S]
    wqT_r = wqT[:].rearrange("p (j c f) -> p j c f", j=NJ, c=KC)
    wkT_r = wkT[:].rearrange("p (j c f) -> p j c f", j=NJ, c=KC)
    wvT_r = wvT[:].rearrange("(c p) o -> p c o", p=128)

    # j-major: kT[j] is first needed at unit 4j, so kproj(1..3) can drip
    # into the late steps where PE otherwise idles against the ACT pace.
    units = [(j, m) for j in range(NJ) for m in range(NMC)]

    with tile.TileContext(nc) as tc, ExitStack() as ctx:
        consts = ctx.enter_context(tc.tile_pool(name="consts", bufs=1))
        bqkm_sb = consts.tile([128, 2 * (OC // 128) + NT], f32)
        bq_sb = bqkm_sb[:, 0:4]
        bk_sb = bqkm_sb[:, 4:8]
        mask_sb = bqkm_sb[:, 8:8 + NT]
        eshift_sb = consts.tile([128, 1], f32)
        nc.vector.memset(eshift_sb, -4.0)
        if has_bv:
            bv_sb = consts.tile([1, OC], f16)
            ones_sb = consts.tile([1, 128], f16)
            nc.vector.memset(ones_sb, 1.0)

        # Persistent activations
        qkv = ctx.enter_context(tc.tile_pool(name="qkv", bufs=1))
        qT_sb = qkv.tile([128, NJ, S], f16)          # [d-pair, j, pos]
        kT_sb = qkv.tile([128, NJ, S], f16)
        v_sb = qkv.tile([128, NT, HPC, 65], f16)     # [key, tile, head, d+den]

        xw = ctx.enter_context(tc.tile_pool(name="xw", bufs=1))
        wk_sb = xw.tile([128, NJ, KC, 128], f16)
        wq_sb = xw.tile([128, NJ, KC, 128], f16)
        wv_sb = xw.tile([128, KC, OC], f16)
        xs = [xw.tile([128, KC, MC], f16, name=f"xs{s}") for s in range(NMC)]

        # DMA prologue, in first-use order, gating pieces split small:
        # kproj(0,0)'s MM for contraction chunk i needs only wk[:, i, 0:128]
        # and xs[0][:, i, :], so interleave those pieces, smallest first.
        for lo, hi in ((0, 3), (3, 6), (6, 8)):
            nc.sync.dma_start(out=wk_sb[:, 0, lo:hi, :],
                              in_=wkT_r[:, 0, lo:hi, :])
            nc.sync.dma_start(out=xs[0][:, lo:hi, :],
                              in_=xT_r[:, lo:hi, 0:MC])
        nc.sync.dma_start(out=wq_sb[:, 0], in_=wqT_r[:, 0])
        nc.sync.dma_start(out=bqkm_sb, in_=bqkmT[:])
        nc.sync.dma_start(out=xs[1][:, 0:4, :], in_=xT_r[:, 0:4, MC:2 * MC])
        nc.sync.dma_start(out=xs[1][:, 4:8, :], in_=xT_r[:, 4:8, MC:2 * MC])
        for h in range(4):
            nc.sync.dma_start(out=wv_sb[:, 2 * h:2 * h + 2, :],
                              in_=wvT_r[:, 2 * h:2 * h + 2, :])
        for s in range(2, NMC):
            for lo, hi in ((0, 4), (4, 8)):
                nc.sync.dma_start(out=xs[s][:, lo:hi, :],
                                  in_=xT_r[:, lo:hi, s * MC:(s + 1) * MC])
        nc.sync.dma_start(out=wk_sb[:, 1:NJ], in_=wkT_r[:, 1:NJ])
        nc.sync.dma_start(out=wq_sb[:, 1:NJ], in_=wqT_r[:, 1:NJ])
        if has_bv:
            nc.sync.dma_start(out=bv_sb, in_=bv[:])

        with tc.tile_pool(name="exp", bufs=5) as epool, \
             tc.tile_pool(name="csb", bufs=2) as cspool, \
             tc.tile_pool(name="pps", bufs=2, space="PSUM") as ppsum, \
             tc.tile_pool(name="sps", bufs=2, space="PSUM") as spsum, \
             tc.tile_pool(name="cps", bufs=1, space="PSUM") as cpsum:

            # denominator weights: exp(mask) column per head
            for gt in range(NT):
                nc.vector.tensor_copy(
                    out=v_sb[:, gt, :, 64:65],
                    in_=mask_sb[:, gt:gt + 1].to_broadcast([128, HPC, 1]))

            # Projections are emitted in two 4-matmul halves around each
            # step's score matmuls so PE load inside a step stays level.
            def kproj(j, s, half, cell):
                lo, hi = (0, KC // 2) if half == 0 else (KC // 2, KC)
                if half == 0:
                    cell["ps"] = ppsum.tile([128, MC], f32, tag="pp",
                                            name="psk")
                psk = cell["ps"]
                for i in range(lo, hi):
                    nc.tensor.matmul(
                        psk, wk_sb[:, j, i, :],
                        xs[s][:, i, :], start=(i == 0), stop=(i == KC - 1),
                        skip_group_check=True)
                if half == 1:
                    nc.vector.tensor_scalar_add(
                        kT_sb[:, j, s * MC:(s + 1) * MC], psk,
                        bk_sb[:, j:j + 1])

            def qproj(j, m, half, cell):
                lo, hi = (0, KC // 2) if half == 0 else (KC // 2, KC)
                if half == 0:
                    cell["ps"] = ppsum.tile([128, MC], f32, tag="pp",
                                            name="psq")
                psq = cell["ps"]
                for i in range(lo, hi):
                    nc.tensor.matmul(
                        psq, wq_sb[:, j, i, :],
                        xs[m][:, i, :], start=(i == 0), stop=(i == KC - 1),
                        skip_group_check=True)
                if half == 1:
                    nc.vector.tensor_scalar_add(
                        qT_sb[:, j, m * MC:(m + 1) * MC], psq,
                        bq_sb[:, j:j + 1])

            def vproj(gt, half, cell):
                lo, hi = (0, KC // 2) if half == 0 else (KC // 2, KC)
                if half == 0:
                    cell["ps"] = ppsum.tile([128, OC], f32, tag="pp",
                                            name="psv")
                psv = cell["ps"]
                for i in range(lo, hi):
                    nc.tensor.matmul(
                        psv, xs[gt // 4][:, i, (gt % 4) * 128:(gt % 4 + 1) * 128],
                        wv_sb[:, i, :], start=(i == 0),
                        stop=(i == KC - 1 and not has_bv),
                        skip_group_check=True)
                if half == 1:
                    if has_bv:
                        nc.tensor.matmul(psv, ones_sb, bv_sb,
                                         start=False, stop=True,
                                         skip_group_check=True)
                    nc.vector.tensor_scalar_mul(
                        v_sb[:, gt, :, 0:64],
                        psv.rearrange("p (h d) -> p h d", h=HPC),
                        mask_sb[:, gt:gt + 1])

            # Projection drip queue ordered by due step (the step before
            # whose score/context matmuls need the result).  Scores at unit
            # u=4j+m, step i consume kT keys < 256(i+1) (kproj chunk s covers
            # keys [512s, 512s+512)) and qT[j, m]; ctx at step n+2 consumes
            # v tiles 2(n%8), 2(n%8)+1.
            tasks = []   # (due_step, fn, args)
            for j in range(NJ):
                for s in range(NMC):
                    if (j, s) != (0, 0):
                        tasks.append((8 * 4 * j + 2 * s - 1, kproj, (j, s)))
            for j in range(NJ):
                for m in range(NMC):
                    if (j, m) != (0, 0):
                        tasks.append((8 * (4 * j + m) - 1, qproj, (j, m)))
            for gt in range(NT):
                tasks.append((max(gt // 2 + 1, 0), vproj, (gt,)))
            # Backward-pack with spacing: place each task at the latest free
            # step at or before its deadline, at most one per step past the
            # forced front and >=2 steps apart, so the drip fills the late
            # steps where sc+ctx alone run below the ACT pace.
            NSTEP = len(units) * 8              # 128
            slots = {}                          # step -> [task]
            for due, fn, args in sorted(tasks, key=lambda t: -t[0]):
                s = min(due, NSTEP - 1)
                while s > 9 and slots.get(s):
                    s -= 1
                slots.setdefault(s, []).append((fn, args))

            # prologue: just enough for unit (0,0) step 0
            c = {}
            kproj(0, 0, 0, c)
            kproj(0, 0, 1, c)
            c = {}
            qproj(0, 0, 0, c)
            qproj(0, 0, 1, c)

            et_ring = {}                        # step -> (et_h0, et_h1)
            pc_cur = [None, None]               # open ctx psum per head slot

            def ctx_group(n):
                u, i = divmod(n, 8)
                j, m = units[u]
                for hh in range(2):
                    if i == 0:
                        pc_cur[hh] = cpsum.tile([65, MC], f32,
                                                tag=f"ctx{hh}",
                                                name=f"pc{hh}")
                    pc = pc_cur[hh]
                    g = 2 * j + hh
                    et = et_ring[n][hh]
                    for tl in range(2):
                        t = 2 * i + tl
                        nc.tensor.matmul(
                            pc, v_sb[:, t, g, :], et[:, tl, :],
                            start=(t == 0), stop=(t == NT - 1),
                            skip_group_check=True)
                    if i == 7:
                        cs = cspool.tile([65, MC], f32, tag=f"cs{hh}",
                                         name=f"cs{hh}")
                        nc.vector.tensor_copy(out=cs, in_=pc)
                        nc.sync.dma_start(
                            out=out[g * 65:(g + 1) * 65, m * MC:(m + 1) * MC],
                            in_=cs)
                del et_ring[n]

            for step in range(NSTEP):
                u, i = divmod(step, 8)
                j, m = units[u]
                ms = slice(m * MC, (m + 1) * MC)
                # trailing context and drip half-0 go FIRST so PE does
                # useful work while the score psum slot waits on exp's ack;
                # drip half-1 lands after, leveling PE load within the step
                if step >= 4:
                    ctx_group(step - 4)
                pend = []
                if step >= 16:
                    # steady state: drip half-0 absorbs the sc ack-wait
                    for fn, args in slots.get(step, ()):
                        cell = {}
                        fn(*args, 0, cell)
                        pend.append((fn, args, cell))
                # scores: 2 k-tiles x 2 heads
                sc = [spsum.tile([128, 2, MC], f32, tag="sc", name=f"sc{hh}")
                      for hh in range(2)]
                for hh in range(2):
                    for tl in range(2):
                        t = 2 * i + tl
                        nc.tensor.matmul(
                            sc[hh][:, tl, :],
                            kT_sb[hh * 64:(hh + 1) * 64, j,
                                  t * 128:(t + 1) * 128],
                            qT_sb[hh * 64:(hh + 1) * 64, j, ms],
                            start=True, stop=True,
                            tile_position=(hh * 64, 0))
                # exp (constant -4 shift cancels in softmax; guards ranges)
                ets = []
                for hh in range(2):
                    et = epool.tile([128, 2, MC], f16, tag=f"exp{hh}",
                                    name=f"et{hh}")
                    nc.scalar.activation(
                        out=et, in_=sc[hh],
                        func=mybir.ActivationFunctionType.Exp,
                        bias=eshift_sb[:, 0:1])
                    ets.append(et)
                et_ring[step] = ets
                if step < 16:
                    # DMA-bound front: keep independent sc ahead of
                    # DMA-gated projection pieces in the PE FIFO
                    for fn, args in slots.get(step, ()):
                        cell = {}
                        fn(*args, 0, cell)
                        fn(*args, 1, cell)
                else:
                    for fn, args, cell in pend:
                        fn(*args, 1, cell)
            for n in range(NSTEP - 4, NSTEP):
                ctx_group(n)

    nc.finalize()
    return nc


def _get_nc(has_bv: bool):
    key = ("nc", has_bv)
    if key not in _CACHE:
        _CACHE[key] = _build(has_bv)
    return _CACHE[key]


def _prep_in_maps(hidden_states, attention_mask, Wq, bq, Wk, bk, Wv, bv):
    hs = np.ascontiguousarray(np.asarray(hidden_states, dtype=np.float32))
    mask = np.asarray(attention_mask, dtype=np.float32)
    Wq = np.asarray(Wq, dtype=np.float32)
    Wk = np.asarray(Wk, dtype=np.float32)
    Wv = np.asarray(Wv, dtype=np.float32)
    bq = np.asarray(bq, dtype=np.float32)
    bk = np.asarray(bk, dtype=np.float32)
    bv = np.asarray(bv, dtype=np.float32)
    scale = 1.0 / np.sqrt(np.float32(HD))
    has_bv = bool(np.any(bv != 0.0))

    in_maps = []
    for c in range(8):
        b, hg = c // 2, c % 2
        sl = slice(hg * OC, (hg + 1) * OC)
        bqkm = np.concatenate([
            (bq[sl] * scale).reshape(OC // 128, 128).T,
            bk[sl].reshape(OC // 128, 128).T,
            np.exp(mask[b]).reshape(NT, 128).T,
        ], axis=1).astype(np.float32)
        m = {
            "xt": np.ascontiguousarray(hs[b].T.astype(np.float16)),
            "wqt": np.ascontiguousarray(
                (Wq[sl] * scale).T.astype(np.float16).reshape(
                    KC, 128, NJ, 128).transpose(1, 2, 0, 3).reshape(
                    128, NJ * KC * 128)),
            "wkt": np.ascontiguousarray(
                Wk[sl].T.astype(np.float16).reshape(
                    KC, 128, NJ, 128).transpose(1, 2, 0, 3).reshape(
                    128, NJ * KC * 128)),
            "wvt": np.ascontiguousarray(Wv[sl].T.astype(np.float16)),
            "bqkm": np.ascontiguousarray(bqkm),
        }
        if has_bv:
            m["bv"] = np.ascontiguousarray(bv[sl].reshape(1, OC).astype(np.float16))
        in_maps.append(m)
    return in_maps, has_bv


def kernel(hidden_states, attention_mask, Wq, bq, Wk, bk, Wv, bv):
    from concourse import bass_utils

    in_maps, has_bv = _prep_in_maps(
        hidden_states, attention_mask, Wq, bq, Wk, bk, Wv, bv)
    nc = _get_nc(has_bv)
    res = bass_utils.run_bass_kernel_spmd(nc, in_maps, core_ids=list(range(8)))
    full = np.empty((B, S, H), dtype=np.float32)
    for c in range(8):
        b, hg = c // 2, c % 2
        o = res.results[c]["out"].reshape(HPC, 65, S)
        ctx = o[:, :64, :] / o[:, 64:65, :]             # softmax denominator
        full[b, :, hg * OC:(hg + 1) * OC] = \
            ctx.transpose(2, 0, 1).reshape(S, OC)
    return full


# revision 61
# speedup vs baseline: 282.9690x; 1.0009x over previous
"""BertSelfAttention kernel for Trainium2 (Bass/Tile), 8-core SPMD.

Full inputs in, full output out. Sharding: core c handles batch b = c//2 and
head-group hg = c%2 (8 of the 16 heads). Each core computes q/k/v projections
for its 512 features and full attention for its 8 heads; the host assembles
out[b, :, hg*512:(hg+1)*512] from each core. No collectives.

Structure (per core): a single 128-step software pipeline; step n emits
  - 4 score matmuls (2 k-tiles x 2 heads of the current head-pair, f16,
    head-packed via tile_position)
  - 2 exp activations (ACT, psum->sbuf f16, FD 1024) for the k-tile pair
  - 4 context matmuls trailing 4 steps (f16, M=65: the 65th stationary
    column is exp(attention_mask), so the softmax denominator accumulates
    alongside the context rows for free)
  - projection matmul half-groups (q/k/v, f16), backward-packed to the
    latest step allowed by their deadline; units iterate j-major so the
    k-projections of later head-pairs fill the late steps where scores +
    context alone run below the ACT exp pace
Context+denominator PSUM tiles are copied to SBUF and DMAed out
feature-major; the host divides by the denominator row and transposes.
PE runs ~300us of matmul (the bound), ACT ~266us of exp; DVE/DMA hide
underneath. TimelineSim: ~316us vs 372us for the previous kernel.

Problem shapes (hardcoded): B=4, S=2048, H=1024, nh=16, hd=64.
"""

import numpy as np

B, S, H = 4, 2048, 1024
NH, HD = 16, 64
HPC = 8          # heads per core
OC = HPC * HD    # output features per core (512)
NT = S // 128    # key tiles (16)
MC = 512         # m chunk (q positions per unit)
NMC = S // MC    # 4
KC = H // 128    # contraction chunks for projections (8)
NJ = HPC // 2    # head pairs (4)
OROW = HPC * 65  # output rows: per head 64 ctx features + 1 denominator

_CACHE = {}


def _build(has_bv: bool):
    from contextlib import ExitStack

    import concourse.bass as bass
    from concourse import bacc
    import concourse.tile as tile
    from concourse import mybir

    f32 = mybir.dt.float32
    f16 = mybir.dt.float16

    nc = bacc.Bacc(trn_type="TRN2")

    xT = nc.dram_tensor("xt", [H, S], f16, kind="ExternalInput")
    # [128, NJ, KC, 128]: per-head-pair column blocks contiguous per
    # partition row, so block DMAs move 2KB descriptors (full bandwidth)
    wqT = nc.dram_tensor("wqt", [128, NJ * KC * 128], f16,
                         kind="ExternalInput")
    wkT = nc.dram_tensor("wkt", [128, NJ * KC * 128], f16,
                         kind="ExternalInput")
    wvT = nc.dram_tensor("wvt", [H, OC], f16, kind="ExternalInput")
    # packed [bq(4) | bk(4) | exp(mask)(16)] to keep the DMA prologue short
    bqkmT = nc.dram_tensor("bqkm", [128, 2 * (OC // 128) + NT], f32,
                           kind="ExternalInput")
    if has_bv:
        bv = nc.dram_tensor("bv", [1, OC], f16, kind="ExternalInput")
    out = nc.dram_tensor("out", [OROW, S], f32, kind="ExternalOutput")

    xT_r = xT[:].rearrange("(c p) s -> p c s", p=128)      # [128, KC, S]
    wqT_r = wqT[:].rearrange("p (j c f) -> p j c f", j=NJ, c=KC)
    wkT_r = wkT[:].rearrange("p (j c f) -> p j c f", j=NJ, c=KC)
    wvT_r = wvT[:].rearrange("(c p) o -> p c o", p=128)

    # j-major: kT[j] is first needed at unit 4j, so kproj(1..3) can drip
    # into the late steps where PE otherwise idles against the ACT pace.
    units = [(j, m) for j in range(NJ) for m in range(NMC)]

    with tile.TileContext(nc) as tc, ExitStack() as ctx:
        consts = ctx.enter_context(tc.tile_pool(name="consts", bufs=1))
        bqkm_sb = consts.tile([128, 2 * (OC // 128) + NT], f32)
        bq_sb = bqkm_sb[:, 0:4]
        bk_sb = bqkm_sb[:, 4:8]
        mask_sb = bqkm_sb[:, 8:8 + NT]
        eshift_sb = consts.tile([128, 1], f32)
        nc.vector.memset(eshift_sb, -4.0)
        if has_bv:
            bv_sb = consts.tile([1, OC], f16)
            ones_sb = consts.tile([1, 128], f16)
            nc.vector.memset(ones_sb, 1.0)

        # Persistent activations
        qkv = ctx.enter_context(tc.tile_pool(name="qkv", bufs=1))
        qT_sb = qkv.tile([128, NJ, S], f16)          # [d-pair, j, pos]
        kT_sb = qkv.tile([128, NJ, S], f16)
        v_sb = qkv.tile([128, NT, HPC, 65], f16)     # [key, tile, head, d+den]

        xw = ctx.enter_context(tc.tile_pool(name="xw", bufs=1))
        wk_sb = xw.tile([128, NJ, KC, 128], f16)
        wq_sb = xw.tile([128, NJ, KC, 128], f16)
        wv_sb = xw.tile([128, KC, OC], f16)
        xs = [xw.tile([128, KC, MC], f16, name=f"xs{s}") for s in range(NMC)]

        # DMA prologue, in first-use order, gating pieces split small:
        # kproj(0,0)'s MM for contraction chunk i needs only wk[:, i, 0:128]
        # and xs[0][:, i, :], so interleave those pieces, smallest first.
        for lo, hi in ((0, 3), (3, 6), (6, 8)):
            nc.sync.dma_start(out=wk_sb[:, 0, lo:hi, :],
                              in_=wkT_r[:, 0, lo:hi, :])
            nc.sync.dma_start(out=xs[0][:, lo:hi, :],
                              in_=xT_r[:, lo:hi, 0:MC])
        nc.sync.dma_start(out=wq_sb[:, 0], in_=wqT_r[:, 0])
        nc.sync.dma_start(out=bqkm_sb, in_=bqkmT[:])
        nc.sync.dma_start(out=xs[1][:, 0:4, :], in_=xT_r[:, 0:4, MC:2 * MC])
        nc.sync.dma_start(out=xs[1][:, 4:8, :], in_=xT_r[:, 4:8, MC:2 * MC])
        for h in range(4):
            nc.sync.dma_start(out=wv_sb[:, 2 * h:2 * h + 2, :],
                              in_=wvT_r[:, 2 * h:2 * h + 2, :])
        for s in range(2, NMC):
            for lo, hi in ((0, 4), (4, 8)):
                nc.sync.dma_start(out=xs[s][:, lo:hi, :],
                                  in_=xT_r[:, lo:hi, s * MC:(s + 1) * MC])
        nc.sync.dma_start(out=wk_sb[:, 1:NJ], in_=wkT_r[:, 1:NJ])
        nc.sync.dma_start(out=wq_sb[:, 1:NJ], in_=wqT_r[:, 1:NJ])
        if has_bv:
            nc.sync.dma_start(out=bv_sb, in_=bv[:])

        with tc.tile_pool(name="exp", bufs=5) as epool, \
             tc.tile_pool(name="csb", bufs=2) as cspool, \
             tc.tile_pool(name="pps", bufs=2, space="PSUM") as ppsum, \
             tc.tile_pool(name="sps", bufs=2, space="PSUM") as spsum, \
             tc.tile_pool(name="cps", bufs=1, space="PSUM") as cpsum:

            # denominator weights: exp(mask) column per head
            for gt in range(NT):
                nc.vector.tensor_copy(
                    out=v_sb[:, gt, :, 64:65],
                    in_=mask_sb[:, gt:gt + 1].to_broadcast([128, HPC, 1]))

            # Projections are emitted in two 4-matmul halves around each
            # step's score matmuls so PE load inside a step stays level.
            def kproj(j, s, half, cell):
                lo, hi = (0, KC // 2) if half == 0 else (KC // 2, KC)
                if half == 0:
                    cell["ps"] = ppsum.tile([128, MC], f32, tag="pp",
                                            name="psk")
                psk = cell["ps"]
                for i in range(lo, hi):
                    nc.tensor.matmul(
                        psk, wk_sb[:, j, i, :],
                        xs[s][:, i, :], start=(i == 0), stop=(i == KC - 1),
                        skip_group_check=True)
                if half == 1:
                    nc.vector.tensor_scalar_add(
                        kT_sb[:, j, s * MC:(s + 1) * MC], psk,
                        bk_sb[:, j:j + 1])

            def qproj(j, m, half, cell):
                lo, hi = (0, KC // 2) if half == 0 else (KC // 2, KC)
                if half == 0:
                    cell["ps"] = ppsum.tile([128, MC], f32, tag="pp",
                                            name="psq")
                psq = cell["ps"]
                for i in range(lo, hi):
                    nc.tensor.matmul(
                        psq, wq_sb[:, j, i, :],
                        xs[m][:, i, :], start=(i == 0), stop=(i == KC - 1),
                        skip_group_check=True)
                if half == 1:
                    nc.vector.tensor_scalar_add(
                        qT_sb[:, j, m * MC:(m + 1) * MC], psq,
                        bq_sb[:, j:j + 1])

            def vproj(gt, half, cell):
                lo, hi = (0, KC // 2) if half == 0 else (KC // 2, KC)
                if half == 0:
                    cell["ps"] = ppsum.tile([128, OC], f32, tag="pp",
                                            name="psv")
                psv = cell["ps"]
                for i in range(lo, hi):
                    nc.tensor.matmul(
                        psv, xs[gt // 4][:, i, (gt % 4) * 128:(gt % 4 + 1) * 128],
                        wv_sb[:, i, :], start=(i == 0),
                        stop=(i == KC - 1 and not has_bv),
                        skip_group_check=True)
                if half == 1:
                    if has_bv:
                        nc.tensor.matmul(psv, ones_sb, bv_sb,
                                         start=False, stop=True,
                                         skip_group_check=True)
                    nc.vector.tensor_scalar_mul(
                        v_sb[:, gt, :, 0:64],
                        psv.rearrange("p (h d) -> p h d", h=HPC),
                        mask_sb[:, gt:gt + 1])

            # Projection drip queue ordered by due step (the step before
            # whose score/context matmuls need the result).  Scores at unit
            # u=4j+m, step i consume kT keys < 256(i+1) (kproj chunk s covers
            # keys [512s, 512s+512)) and qT[j, m]; ctx at step n+2 consumes
            # v tiles 2(n%8), 2(n%8)+1.
            tasks = []   # (due_step, fn, args)
            for j in range(NJ):
                for s in range(NMC):
                    if (j, s) != (0, 0):
                        tasks.append((8 * 4 * j + 2 * s - 1, kproj, (j, s)))
            for j in range(NJ):
                for m in range(NMC):
                    if (j, m) != (0, 0):
                        tasks.append((8 * (4 * j + m) - 1, qproj, (j, m)))
            for gt in range(NT):
                tasks.append((max(gt // 2 + 1, 0), vproj, (gt,)))
            # Backward-pack with spacing: place each task at the latest free
            # step at or before its deadline, at most one per step past the
            # forced front and >=2 steps apart, so the drip fills the late
            # steps where sc+ctx alone run below the ACT pace.
            NSTEP = len(units) * 8              # 128
            slots = {}                          # step -> [task]
            for due, fn, args in sorted(tasks, key=lambda t: -t[0]):
                s = min(due, NSTEP - 1)
                while s > 9 and slots.get(s):
                    s -= 1
                slots.setdefault(s, []).append((fn, args))

            # prologue: just enough for unit (0,0) step 0
            c = {}
            kproj(0, 0, 0, c)
            kproj(0, 0, 1, c)
            c = {}
            qproj(0, 0, 0, c)
            qproj(0, 0, 1, c)

            et_ring = {}                        # step -> (et_h0, et_h1)
            pc_cur = [None, None]               # open ctx psum per head slot

            def ctx_group(n):
                u, i = divmod(n, 8)
                j, m = units[u]
                for hh in range(2):
                    if i == 0:
                        pc_cur[hh] = cpsum.tile([65, MC], f32,
                                                tag=f"ctx{hh}",
                                                name=f"pc{hh}")
                    pc = pc_cur[hh]
                    g = 2 * j + hh
                    et = et_ring[n][hh]
                    for tl in range(2):
                        t = 2 * i + tl
                        nc.tensor.matmul(
                            pc, v_sb[:, t, g, :], et[:, tl, :],
                            start=(t == 0), stop=(t == NT - 1),
                            skip_group_check=True)
                    if i == 7:
                        cs = cspool.tile([65, MC], f32, tag=f"cs{hh}",
                                         name=f"cs{hh}")
                        nc.vector.tensor_copy(out=cs, in_=pc)
                        nc.sync.dma_start(
                            out=out[g * 65:(g + 1) * 65, m * MC:(m + 1) * MC],
                            in_=cs)
                del et_ring[n]

            for step in range(NSTEP):
                u, i = divmod(step, 8)
                j, m = units[u]
                ms = slice(m * MC, (m + 1) * MC)
                # trailing context and drip half-0 go FIRST so PE does
                # useful work while the score psum slot waits on exp's ack;
                # drip half-1 lands after, leveling PE load within the step
                if step >= 4:
                    ctx_group(step - 4)
                pend = []
                if step >= 16:
                    # steady state: drip half-0 absorbs the sc ack-wait
                    for fn, args in slots.get(step, ()):
                        cell = {}
                        fn(*args, 0, cell)
                        pend.append((fn, args, cell))
                # scores: 2 k-tiles x 2 heads
                sc = [spsum.tile([128, 2, MC], f32, tag="sc", name=f"sc{hh}")
                      for hh in range(2)]
                for hh in range(2):
                    for tl in range(2):
                        t = 2 * i + tl
                        nc.tensor.matmul(
                            sc[hh][:, tl, :],
                            kT_sb[hh * 64:(hh + 1) * 64, j,
                                  t * 128:(t + 1) * 128],
                            qT_sb[hh * 64:(hh + 1) * 64, j, ms],
                            start=True, stop=True,
                            tile_position=(hh * 64, 0))
                # exp (constant -4 shift cancels in softmax; guards ranges)
                ets = []
                for hh in range(2):
                    et = epool.tile([128, 2, MC], f16, tag=f"exp{hh}",
                                    name=f"et{hh}")
                    nc.scalar.activation(
                        out=et, in_=sc[hh],
                        func=mybir.ActivationFunctionType.Exp,
                        bias=eshift_sb[:, 0:1])
                    ets.append(et)
                et_ring[step] = ets
                if step < 16:
                    # DMA-bound front: keep independent sc ahead of
                    # DMA-gated projection pieces in the PE FIFO
                    for fn, args in slots.get(step, ()):
                        cell = {}
                        fn(*args, 0, cell)
                        fn(*args, 1, cell)
                else:
                    for fn, args, cell in pend:
                        fn(*args, 1, cell)
            for n in range(NSTEP - 4, NSTEP):
                ctx_group(n)

    nc.finalize()
    return nc


def _get_nc(has_bv: bool):
    key = ("nc", has_bv)
    if key not in _CACHE:
        _CACHE[key] = _build(has_bv)
    return _CACHE[key]


def _prep_in_maps(hidden_states, attention_mask, Wq, bq, Wk, bk, Wv, bv):
    hs = np.ascontiguousarray(np.asarray(hidden_states, dtype=np.float32))
    mask = np.asarray(attention_mask, dtype=np.float32)
    Wq = np.asarray(Wq, dtype=np.float32)
    Wk = np.asarray(Wk, dtype=np.float32)
    Wv = np.asarray(Wv, dtype=np.float32)
    bq = np.asarray(bq, dtype=np.float32)
    bk = np.asarray(bk, dtype=np.float32)
    bv = np.asarray(bv, dtype=np.float32)
    scale = 1.0 / np.sqrt(np.float32(HD))
    has_bv = bool(np.any(bv != 0.0))

    in_maps = []
    for c in range(8):
        b, hg = c // 2, c % 2
        sl = slice(hg * OC, (hg + 1) * OC)
        bqkm = np.concatenate([
            (bq[sl] * scale).reshape(OC // 128, 128).T,
            bk[sl].reshape(OC // 128, 128).T,
            np.exp(mask[b]).reshape(NT, 128).T,
        ], axis=1).astype(np.float32)
        m = {
            "xt": np.ascontiguousarray(hs[b].T.astype(np.float16)),
            "wqt": np.ascontiguousarray(
                (Wq[sl] * scale).T.astype(np.float16).reshape(
                    KC, 128, NJ, 128).transpose(1, 2, 0, 3).reshape(
                    128, NJ * KC * 128)),
            "wkt": np.ascontiguousarray(
                Wk[sl].T.astype(np.float16).reshape(
                    KC, 128, NJ, 128).transpose(1, 2, 0, 3).reshape(
                    128, NJ * KC * 128)),
            "wvt": np.ascontiguousarray(Wv[sl].T.astype(np.float16)),
            "bqkm": np.ascontiguousarray(bqkm),
        }
        if has_bv:
            m["bv"] = np.ascontiguousarray(bv[sl].reshape(1, OC).astype(np.float16))
        in_maps.append(m)
    return in_maps, has_bv


def kernel(hidden_states, attention_mask, Wq, bq, Wk, bk, Wv, bv):
    from concourse import bass_utils

    in_maps, has_bv = _prep_in_maps(
        hidden_states, attention_mask, Wq, bq, Wk, bk, Wv, bv)
    nc = _get_nc(has_bv)
    res = bass_utils.run_bass_kernel_spmd(nc, in_maps, core_ids=list(range(8)))
    full = np.empty((B, S, H), dtype=np.float32)
    for c in range(8):
        b, hg = c // 2, c % 2
        o = res.results[c]["out"].reshape(HPC, 65, S)
        ctx = o[:, :64, :] / o[:, 64:65, :]             # softmax denominator
        full[b, :, hg * OC:(hg + 1) * OC] = \
            ctx.transpose(2, 0, 1).reshape(S, OC)
    return full


# revision 63
# speedup vs baseline: 283.0339x; 1.0002x over previous
"""BertSelfAttention kernel for Trainium2 (Bass/Tile), 8-core SPMD.

Full inputs in, full output out. Sharding: core c handles batch b = c//2 and
head-group hg = c%2 (8 of the 16 heads). Each core computes q/k/v projections
for its 512 features and full attention for its 8 heads; the host assembles
out[b, :, hg*512:(hg+1)*512] from each core. No collectives.

Structure (per core): a single 128-step software pipeline; step n emits
  - 4 score matmuls (2 k-tiles x 2 heads of the current head-pair, f16,
    head-packed via tile_position)
  - 2 exp activations (ACT, psum->sbuf f16, FD 1024) for the k-tile pair
  - 4 context matmuls trailing 4 steps (f16, M=65: the 65th stationary
    column is exp(attention_mask), so the softmax denominator accumulates
    alongside the context rows for free)
  - projection matmul half-groups (q/k/v, f16), backward-packed to the
    latest step allowed by their deadline; units iterate j-major so the
    k-projections of later head-pairs fill the late steps where scores +
    context alone run below the ACT exp pace
Context+denominator PSUM tiles are copied to SBUF and DMAed out
feature-major; the host divides by the denominator row and transposes.
PE runs ~300us of matmul (the bound), ACT ~266us of exp; DVE/DMA hide
underneath. TimelineSim: ~316us vs 372us for the previous kernel.

Problem shapes (hardcoded): B=4, S=2048, H=1024, nh=16, hd=64.
"""

import numpy as np

B, S, H = 4, 2048, 1024
NH, HD = 16, 64
HPC = 8          # heads per core
OC = HPC * HD    # output features per core (512)
NT = S // 128    # key tiles (16)
MC = 512         # m chunk (q positions per unit)
NMC = S // MC    # 4
KC = H // 128    # contraction chunks for projections (8)
NJ = HPC // 2    # head pairs (4)
OROW = HPC * 65  # output rows: per head 64 ctx features + 1 denominator

_CACHE = {}


def _build(has_bv: bool):
    from contextlib import ExitStack

    import concourse.bass as bass
    from concourse import bacc
    import concourse.tile as tile
    from concourse import mybir

    f32 = mybir.dt.float32
    f16 = mybir.dt.float16

    nc = bacc.Bacc(trn_type="TRN2")

    xT = nc.dram_tensor("xt", [H, S], f16, kind="ExternalInput")
    # [128, NJ, KC, 128]: per-head-pair column blocks contiguous per
    # partition row, so block DMAs move 2KB descriptors (full bandwidth)
    wqT = nc.dram_tensor("wqt", [128, NJ * KC * 128], f16,
                         kind="ExternalInput")
    wkT = nc.dram_tensor("wkt", [128, NJ * KC * 128], f16,
                         kind="ExternalInput")
    wvT = nc.dram_tensor("wvt", [H, OC], f16, kind="ExternalInput")
    # packed [bq(4) | bk(4) | exp(mask)(16)] to keep the DMA prologue short
    bqkmT = nc.dram_tensor("bqkm", [128, 2 * (OC // 128) + NT], f32,
                           kind="ExternalInput")
    if has_bv:
        bv = nc.dram_tensor("bv", [1, OC], f16, kind="ExternalInput")
    out = nc.dram_tensor("out", [OROW, S], f32, kind="ExternalOutput")

    xT_r = xT[:].rearrange("(c p) s -> p c s", p=128)      # [128, KC, S]
    wqT_r = wqT[:].rearrange("p (j c f) -> p j c f", j=NJ, c=KC)
    wkT_r = wkT[:].rearrange("p (j c f) -> p j c f", j=NJ, c=KC)
    wvT_r = wvT[:].rearrange("(c p) o -> p c o", p=128)

    # j-major: kT[j] is first needed at unit 4j, so kproj(1..3) can drip
    # into the late steps where PE otherwise idles against the ACT pace.
    units = [(j, m) for j in range(NJ) for m in range(NMC)]

    with tile.TileContext(nc) as tc, ExitStack() as ctx:
        consts = ctx.enter_context(tc.tile_pool(name="consts", bufs=1))
        bqkm_sb = consts.tile([128, 2 * (OC // 128) + NT], f32)
        bq_sb = bqkm_sb[:, 0:4]
        bk_sb = bqkm_sb[:, 4:8]
        mask_sb = bqkm_sb[:, 8:8 + NT]
        eshift_sb = consts.tile([128, 1], f32)
        nc.vector.memset(eshift_sb, -4.0)
        if has_bv:
            bv_sb = consts.tile([1, OC], f16)
            ones_sb = consts.tile([1, 128], f16)
            nc.vector.memset(ones_sb, 1.0)

        # Persistent activations
        qkv = ctx.enter_context(tc.tile_pool(name="qkv", bufs=1))
        qT_sb = qkv.tile([128, NJ, S], f16)          # [d-pair, j, pos]
        kT_sb = qkv.tile([128, NJ, S], f16)
        v_sb = qkv.tile([128, NT, HPC, 65], f16)     # [key, tile, head, d+den]

        xw = ctx.enter_context(tc.tile_pool(name="xw", bufs=1))
        wk_sb = xw.tile([128, NJ, KC, 128], f16)
        wq_sb = xw.tile([128, NJ, KC, 128], f16)
        wv_sb = xw.tile([128, KC, OC], f16)
        xs = [xw.tile([128, KC, MC], f16, name=f"xs{s}") for s in range(NMC)]

        # DMA prologue, in first-use order, gating pieces split small:
        # kproj(0,0)'s MM for contraction chunk i needs only wk[:, i, 0:128]
        # and xs[0][:, i, :], so interleave those pieces, smallest first.
        for lo, hi in ((0, 3), (3, 6), (6, 8)):
            nc.sync.dma_start(out=wk_sb[:, 0, lo:hi, :],
                              in_=wkT_r[:, 0, lo:hi, :])
            nc.sync.dma_start(out=xs[0][:, lo:hi, :],
                              in_=xT_r[:, lo:hi, 0:MC])
        nc.sync.dma_start(out=wq_sb[:, 0], in_=wqT_r[:, 0])
        nc.sync.dma_start(out=bqkm_sb, in_=bqkmT[:])
        nc.sync.dma_start(out=xs[1][:, 0:4, :], in_=xT_r[:, 0:4, MC:2 * MC])
        nc.sync.dma_start(out=xs[1][:, 4:8, :], in_=xT_r[:, 4:8, MC:2 * MC])
        for h in range(4):
            nc.sync.dma_start(out=wv_sb[:, 2 * h:2 * h + 2, :],
                              in_=wvT_r[:, 2 * h:2 * h + 2, :])
        for s in range(2, NMC):
            for lo, hi in ((0, 4), (4, 8)):
                nc.sync.dma_start(out=xs[s][:, lo:hi, :],
                                  in_=xT_r[:, lo:hi, s * MC:(s + 1) * MC])
        nc.sync.dma_start(out=wk_sb[:, 1:NJ], in_=wkT_r[:, 1:NJ])
        nc.sync.dma_start(out=wq_sb[:, 1:NJ], in_=wqT_r[:, 1:NJ])
        if has_bv:
            nc.sync.dma_start(out=bv_sb, in_=bv[:])

        with tc.tile_pool(name="exp", bufs=5) as epool, \
             tc.tile_pool(name="csb", bufs=2) as cspool, \
             tc.tile_pool(name="pps", bufs=2, space="PSUM") as ppsum, \
             tc.tile_pool(name="sps", bufs=2, space="PSUM") as spsum, \
             tc.tile_pool(name="cps", bufs=1, space="PSUM") as cpsum:

            # denominator weights: exp(mask) column per head
            for gt in range(NT):
                nc.vector.tensor_copy(
                    out=v_sb[:, gt, :, 64:65],
                    in_=mask_sb[:, gt:gt + 1].to_broadcast([128, HPC, 1]))

            # Projections are emitted in two 4-matmul halves around each
            # step's score matmuls so PE load inside a step stays level.
            def kproj(j, s, half, cell):
                lo, hi = (0, KC // 2) if half == 0 else (KC // 2, KC)
                if half == 0:
                    cell["ps"] = ppsum.tile([128, MC], f32, tag="pp",
                                            name="psk")
                psk = cell["ps"]
                for i in range(lo, hi):
                    nc.tensor.matmul(
                        psk, wk_sb[:, j, i, :],
                        xs[s][:, i, :], start=(i == 0), stop=(i == KC - 1),
                        skip_group_check=True)
                if half == 1:
                    nc.vector.tensor_scalar_add(
                        kT_sb[:, j, s * MC:(s + 1) * MC], psk,
                        bk_sb[:, j:j + 1])

            def qproj(j, m, half, cell):
                lo, hi = (0, KC // 2) if half == 0 else (KC // 2, KC)
                if half == 0:
                    cell["ps"] = ppsum.tile([128, MC], f32, tag="pp",
                                            name="psq")
                psq = cell["ps"]
                for i in range(lo, hi):
                    nc.tensor.matmul(
                        psq, wq_sb[:, j, i, :],
                        xs[m][:, i, :], start=(i == 0), stop=(i == KC - 1),
                        skip_group_check=True)
                if half == 1:
                    nc.vector.tensor_scalar_add(
                        qT_sb[:, j, m * MC:(m + 1) * MC], psq,
                        bq_sb[:, j:j + 1])

            def vproj(gt, half, cell):
                lo, hi = (0, KC // 2) if half == 0 else (KC // 2, KC)
                if half == 0:
                    cell["ps"] = ppsum.tile([128, OC], f32, tag="pp",
                                            name="psv")
                psv = cell["ps"]
                for i in range(lo, hi):
                    nc.tensor.matmul(
                        psv, xs[gt // 4][:, i, (gt % 4) * 128:(gt % 4 + 1) * 128],
                        wv_sb[:, i, :], start=(i == 0),
                        stop=(i == KC - 1 and not has_bv),
                        skip_group_check=True)
                if half == 1:
                    if has_bv:
                        nc.tensor.matmul(psv, ones_sb, bv_sb,
                                         start=False, stop=True,
                                         skip_group_check=True)
                    nc.vector.tensor_scalar_mul(
                        v_sb[:, gt, :, 0:64],
                        psv.rearrange("p (h d) -> p h d", h=HPC),
                        mask_sb[:, gt:gt + 1])

            # Projection drip queue ordered by due step (the step before
            # whose score/context matmuls need the result).  Scores at unit
            # u=4j+m, step i consume kT keys < 256(i+1) (kproj chunk s covers
            # keys [512s, 512s+512)) and qT[j, m]; ctx at step n+2 consumes
            # v tiles 2(n%8), 2(n%8)+1.
            tasks = []   # (due_step, fn, args)
            for j in range(NJ):
                for s in range(NMC):
                    if (j, s) != (0, 0):
                        tasks.append((8 * 4 * j + 2 * s - 1, kproj, (j, s)))
            for j in range(NJ):
                for m in range(NMC):
                    if (j, m) != (0, 0):
                        tasks.append((8 * (4 * j + m) - 1, qproj, (j, m)))
            for gt in range(NT):
                tasks.append((max(gt // 2 + 1, 0), vproj, (gt,)))
            # Backward-pack with spacing: place each task at the latest free
            # step at or before its deadline, at most one per step past the
            # forced front and >=2 steps apart, so the drip fills the late
            # steps where sc+ctx alone run below the ACT pace.
            NSTEP = len(units) * 8              # 128
            slots = {}                          # step -> [task]
            for due, fn, args in sorted(tasks, key=lambda t: -t[0]):
                s = min(due, NSTEP - 1)
                while s > 9 and slots.get(s):
                    s -= 1
                slots.setdefault(s, []).append((fn, args))

            # prologue: just enough for unit (0,0) step 0
            c = {}
            kproj(0, 0, 0, c)
            kproj(0, 0, 1, c)
            c = {}
            qproj(0, 0, 0, c)
            qproj(0, 0, 1, c)

            et_ring = {}                        # step -> (et_h0, et_h1)
            pc_cur = [None, None]               # open ctx psum per head slot

            def ctx_group(n):
                u, i = divmod(n, 8)
                j, m = units[u]
                for hh in range(2):
                    if i == 0:
                        pc_cur[hh] = cpsum.tile([65, MC], f32,
                                                tag=f"ctx{hh}",
                                                name=f"pc{hh}")
                    pc = pc_cur[hh]
                    g = 2 * j + hh
                    et = et_ring[n][hh]
                    for tl in range(2):
                        t = 2 * i + tl
                        nc.tensor.matmul(
                            pc, v_sb[:, t, g, :], et[:, tl, :],
                            start=(t == 0), stop=(t == NT - 1),
                            skip_group_check=True)
                    if i == 7:
                        cs = cspool.tile([65, MC], f32, tag=f"cs{hh}",
                                         name=f"cs{hh}")
                        nc.vector.tensor_copy(out=cs, in_=pc)
                        nc.sync.dma_start(
                            out=out[g * 65:(g + 1) * 65, m * MC:(m + 1) * MC],
                            in_=cs)
                del et_ring[n]

            for step in range(NSTEP):
                u, i = divmod(step, 8)
                j, m = units[u]
                ms = slice(m * MC, (m + 1) * MC)
                # trailing context and drip half-0 go FIRST so PE does
                # useful work while the score psum slot waits on exp's ack;
                # drip half-1 lands after, leveling PE load within the step
                if step >= 4:
                    ctx_group(step - 4)
                pend = []
                if step >= 16:
                    # steady state: drip half-0 absorbs the sc ack-wait
                    for fn, args in slots.get(step, ()):
                        cell = {}
                        fn(*args, 0, cell)
                        pend.append((fn, args, cell))
                # scores: 2 k-tiles x 2 heads
                sc = [spsum.tile([128, 2, MC], f32, tag="sc", name=f"sc{hh}")
                      for hh in range(2)]
                for hh in range(2):
                    for tl in range(2):
                        t = 2 * i + tl
                        nc.tensor.matmul(
                            sc[hh][:, tl, :],
                            kT_sb[hh * 64:(hh + 1) * 64, j,
                                  t * 128:(t + 1) * 128],
                            qT_sb[hh * 64:(hh + 1) * 64, j, ms],
                            start=True, stop=True,
                            tile_position=(hh * 64, 0))
                # exp (constant -4 shift cancels in softmax; guards ranges)
                ets = []
                for hh in range(2):
                    et = epool.tile([128, 2, MC], f16, tag=f"exp{hh}",
                                    name=f"et{hh}")
                    nc.scalar.activation(
                        out=et, in_=sc[hh],
                        func=mybir.ActivationFunctionType.Exp,
                        bias=eshift_sb[:, 0:1])
                    ets.append(et)
                et_ring[step] = ets
                if step < 16:
                    # DMA-bound front: keep independent sc ahead of
                    # DMA-gated projection pieces in the PE FIFO
                    for fn, args in slots.get(step, ()):
                        cell = {}
                        fn(*args, 0, cell)
                        fn(*args, 1, cell)
                else:
                    for fn, args, cell in pend:
                        fn(*args, 1, cell)
            for n in range(NSTEP - 4, NSTEP):
                ctx_group(n)

    nc.finalize()
    return nc


def _get_nc(has_bv: bool):
    key = ("nc", has_bv)
    if key not in _CACHE:
        _CACHE[key] = _build(has_bv)
    return _CACHE[key]


def _prep_in_maps(hidden_states, attention_mask, Wq, bq, Wk, bk, Wv, bv):
    hs = np.ascontiguousarray(np.asarray(hidden_states, dtype=np.float32))
    mask = np.asarray(attention_mask, dtype=np.float32)
    Wq = np.asarray(Wq, dtype=np.float32)
    Wk = np.asarray(Wk, dtype=np.float32)
    Wv = np.asarray(Wv, dtype=np.float32)
    bq = np.asarray(bq, dtype=np.float32)
    bk = np.asarray(bk, dtype=np.float32)
    bv = np.asarray(bv, dtype=np.float32)
    scale = 1.0 / np.sqrt(np.float32(HD))
    has_bv = bool(np.any(bv != 0.0))

    in_maps = []
    for c in range(8):
        b, hg = c // 2, c % 2
        sl = slice(hg * OC, (hg + 1) * OC)
        bqkm = np.concatenate([
            (bq[sl] * scale).reshape(OC // 128, 128).T,
            bk[sl].reshape(OC // 128, 128).T,
            np.exp(mask[b]).reshape(NT, 128).T,
        ], axis=1).astype(np.float32)
        m = {
            "xt": np.ascontiguousarray(hs[b].T.astype(np.float16)),
            "wqt": np.ascontiguousarray(
                (Wq[sl] * scale).T.astype(np.float16).reshape(
                    KC, 128, NJ, 128).transpose(1, 2, 0, 3).reshape(
                    128, NJ * KC * 128)),
            "wkt": np.ascontiguousarray(
                Wk[sl].T.astype(np.float16).reshape(
                    KC, 128, NJ, 128).transpose(1, 2, 0, 3).reshape(
                    128, NJ * KC * 128)),
            "wvt": np.ascontiguousarray(Wv[sl].T.astype(np.float16)),
            "bqkm": np.ascontiguousarray(bqkm),
        }
        if has_bv:
            m["bv"] = np.ascontiguousarray(bv[sl].reshape(1, OC).astype(np.float16))
        in_maps.append(m)
    return in_maps, has_bv


def kernel(hidden_states, attention_mask, Wq, bq, Wk, bk, Wv, bv):
    from concourse import bass_utils

    in_maps, has_bv = _prep_in_maps(
        hidden_states, attention_mask, Wq, bq, Wk, bk, Wv, bv)
    nc = _get_nc(has_bv)
    res = bass_utils.run_bass_kernel_spmd(nc, in_maps, core_ids=list(range(8)))
    full = np.empty((B, S, H), dtype=np.float32)
    for c in range(8):
        b, hg = c // 2, c % 2
        o = res.results[c]["out"].reshape(HPC, 65, S)
        ctx = o[:, :64, :] / o[:, 64:65, :]             # softmax denominator
        full[b, :, hg * OC:(hg + 1) * OC] = \
            ctx.transpose(2, 0, 1).reshape(S, OC)
    return full
